# revision 1
# baseline (speedup 1.0000x reference)
"""Trainium2 Bass kernel for nn_ModelAttention2Layers (B=8, S=2048, D=512, K=256).

Key structural insight: the reference returns final[0, -1, :] — only batch 0
matters (attention is independent per batch element), so batches 1-7 are dead
compute. Strategy: shard the 2048-query sequence of batch 0 across the 8
cores (256 queries each), with:
  - block 1 fully local per core (xT replicated -> k1T computed redundantly,
    so block 1 needs zero collectives)
  - one AllGather of the {k2T, v2} shards for block 2
  - block 3 flash-style: tiny AllGather of hidden[-1], per-core partial
    softmax/AV over the local 256 keys, one tiny AllReduce of [o|l]
Matmuls run in float32r (full-rate PE, ~11-bit mantissa); softmax statistics,
normalization and reductions in float32. k-projection biases are dropped —
they shift each query's logits by a per-query constant, which softmax
cancels exactly.
"""
import sys

sys.path.insert(0, "/opt/trn_rl_repo")

import numpy as np

S, D, K, P, C = 2048, 512, 256, 128, 8
SH = S // C          # 256 queries/keys per core
ND, NK, NS, NSH = D // P, K // P, S // P, SH // P   # 4, 2, 16, 2

_cache = {}


def _build():
    import concourse.bass as bass
    import concourse.tile as tile
    from concourse import mybir, bacc

    F32 = mybir.dt.float32
    F32R = mybir.dt.float32r
    BF16 = mybir.dt.bfloat16
    AF = mybir.ActivationFunctionType
    ts = bass.ts

    nc = bacc.Bacc()

    ins = {}
    for name, shape in [
        ("xT", [D, S]), ("x0", [S, D]), ("xTq", [D, SH]),
        ("Wk1", [D, K]), ("Wq1", [D, K]), ("Wk2", [D, K]), ("Wq2", [D, K]),
        ("Wv2", [D, D]), ("bq1", [K]), ("bq2", [K]), ("bv2row", [1, D]),
        ("ones", [1, P]), ("onescol", [P, 1]), ("ident", [P, P]),
    ]:
        ins[name] = nc.dram_tensor(name, shape, F32, kind="ExternalInput")
    out_ext = nc.dram_tensor("out", [D], F32, kind="ExternalOutput")

    GA = NK * P * K + NK * P * D   # gather-A floats per core: k2T + v2 shards

    with tile.TileContext(nc) as tc:
        with tc.tile_pool(name="const", bufs=1) as cw, \
             tc.tile_pool(name="big", bufs=1) as big, \
             tc.tile_pool(name="work", bufs=1) as wk, \
             tc.tile_pool(name="pp", bufs=2) as pp, \
             tc.tile_pool(name="small", bufs=2) as sm, \
             tc.tile_pool(name="stage", bufs=2) as stg, \
             tc.tile_pool(name="ps", bufs=1, space="PSUM") as ps, \
             tc.tile_pool(name="dram", bufs=1, space="DRAM") as dram, \
             tc.tile_pool(name="shdram", bufs=1, space="DRAM") as shd:

            # ---- input loads ----
            # small weights cast-load via gpsimd; bulk tensors (xT, x0) load as
            # f32 on sync HWDGE queues and cast to f32r on DVE (parallel paths,
            # so the first k1T matmul can start within a few us)
            W_r = {}
            for w, ncol in [("Wk1", K), ("Wq1", K)]:
                W_r[w] = cw.tile([P, ND, ncol], F32R, name=f"W_{w}", tag=f"W_{w}")
                nc.gpsimd.dma_start(W_r[w][:], ins[w][:].rearrange("(k p) n -> p k n", p=P))
            xT_r = big.tile([P, ND, S], F32R, tag="XV")
            for k in range(ND):
                st = stg.tile([P, S], F32, tag="stg")
                nc.sync.dma_start(
                    st[:], ins["xT"][:].rearrange("(k2 p) s -> p k2 s", p=P)[:, k, :])
                nc.vector.tensor_copy(xT_r[:, k, :], st[:])
            xTq_r = cw.tile([P, ND, SH], F32R)
            nc.gpsimd.dma_start(xTq_r[:], ins["xTq"][:].rearrange("(k p) j -> p k j", p=P))
            x0_r = cw.tile([P, NS, D], F32R)
            for n4 in range(4):
                st = stg.tile([P, S], F32, tag="stg")
                nc.sync.dma_start(
                    st[:].rearrange("p (n d) -> p n d", n=4),
                    ins["x0"][:].rearrange("(n p) d -> p n d", p=P)[:, 4 * n4:4 * n4 + 4, :])
                nc.vector.tensor_copy(
                    x0_r[:, 4 * n4:4 * n4 + 4, :].rearrange("p n d -> p (n d)"), st[:])
            for w, ncol in [("Wk2", K), ("Wq2", K), ("Wv2", D)]:
                W_r[w] = cw.tile([P, ND, ncol], F32R, name=f"W_{w}", tag=f"W_{w}")
                st = stg.tile([P, ND * ncol], F32, tag="stg", name=f"st_{w}")
                nc.sync.dma_start(
                    st[:].rearrange("p (k n) -> p k n", k=ND),
                    ins[w][:].rearrange("(k p) n -> p k n", p=P))
                nc.vector.tensor_copy(
                    W_r[w][:].rearrange("p k n -> p (k n)"), st[:])
            bq1_sb = cw.tile([P, NK], F32)
            nc.sync.dma_start(bq1_sb[:], ins["bq1"][:].rearrange("(m p) -> p m", p=P))
            bq2_sb = cw.tile([P, NK], F32)
            nc.sync.dma_start(bq2_sb[:], ins["bq2"][:].rearrange("(m p) -> p m", p=P))
            bv2_r = cw.tile([1, D], F32R)
            nc.gpsimd.dma_start(bv2_r[:], ins["bv2row"][:])
            ones_r = cw.tile([1, P], F32R)
            nc.gpsimd.dma_start(ones_r[:], ins["ones"][:])
            ident_r = cw.tile([P, P], F32R)
            nc.gpsimd.dma_start(ident_r[:], ins["ident"][:])
            Wq2f = cw.tile([P, ND, K], F32)
            nc.sync.dma_start(Wq2f[:], ins["Wq2"][:].rearrange("(k p) n -> p k n", p=P))
            onescol_f = cw.tile([P, 1], F32)
            nc.sync.dma_start(onescol_f[:], ins["onescol"][:])

            # ---- block 1 projections ----
            # k1T full [K, S], computed redundantly on every core (no bias: softmax-invariant)
            k1T = big.tile([P, NK, S], F32R, tag="kT")
            for m in range(NK):
                for cb in range(S // 512):
                    pm = ps.tile([P, 512], F32, tag="mm")
                    for k in range(ND):
                        nc.tensor.matmul(pm[:], W_r["Wk1"][:, k, ts(m, P)],
                                         xT_r[:, k, ts(cb, 512)],
                                         start=(k == 0), stop=(k == ND - 1))
                    nc.vector.tensor_copy(k1T[:, m, ts(cb, 512)], pm[:])
            # q1T shard [K, SH] with bias bq1
            q1T = wk.tile([P, NK, SH], F32R, tag="qT")
            for m in range(NK):
                pm = ps.tile([P, SH], F32, tag="mm")
                for k in range(ND):
                    nc.tensor.matmul(pm[:], W_r["Wq1"][:, k, ts(m, P)], xTq_r[:, k, :],
                                     start=(k == 0), stop=(k == ND - 1))
                nc.vector.tensor_scalar_add(q1T[:, m, :], pm[:], bq1_sb[:, m:m + 1])

            def attention(qT, kT_full, V_full, out_dst, pt_dtype):
                """out_dst[:, qm, :] = softmax(q.k^T) @ V for this core's 256 queries."""
                for qm in range(NSH):
                    sc = ps.tile([P, 4, 512], F32, tag="sc")
                    for ks in range(4):
                        for dm in range(NK):
                            nc.tensor.matmul(sc[:, ks, :], qT[:, dm, ts(qm, P)],
                                             kT_full[:, dm, ts(ks, 512)],
                                             start=(dm == 0), stop=(dm == NK - 1))
                    mx = sm.tile([P, 1], F32, tag="mx")
                    nc.vector.reduce_max(mx[:], sc[:], axis=mybir.AxisListType.XY)
                    nm = sm.tile([P, 1], F32, tag="nm")
                    nc.vector.tensor_scalar_mul(nm[:], mx[:], -1.0)
                    Pt = pp.tile([P, S], F32R, tag="P")
                    lsum = sm.tile([P, 4], F32, tag="lsum")
                    for ks in range(4):
                        nc.scalar.activation(Pt[:, ts(ks, 512)], sc[:, ks, :], AF.Exp,
                                             bias=nm[:], accum_out=lsum[:, ks:ks + 1])
                    l = sm.tile([P, 1], F32, tag="l")
                    nc.vector.reduce_sum(l[:], lsum[:], axis=mybir.AxisListType.X)
                    rl = sm.tile([P, 1], F32, tag="rl")
                    nc.vector.reciprocal(rl[:], l[:])
                    PT = pp.tile([P, NS, P], pt_dtype, tag="PT")
                    for n in range(NS):
                        tp = ps.tile([P, P], F32R, tag="tp")
                        nc.tensor.transpose(tp[:], Pt[:, ts(n, P)], ident_r[:])
                        nc.vector.tensor_copy(PT[:, n, :], tp[:])
                    av = ps.tile([P, D], F32, tag="mm")
                    for n in range(NS):
                        nc.tensor.matmul(av[:], PT[:, n, :], V_full[:, n, :],
                                         start=(n == 0), stop=(n == NS - 1))
                    nc.scalar.activation(out_dst[:, qm, :], av[:], AF.Copy, scale=rl[:])

            out1 = wk.tile([P, NSH, D], F32R, tag="H")
            attention(q1T, k1T, x0_r, out1, F32R)

            def transpose_rows(src, ncols_chunks):
                """src [P, NSH, D] -> dst [P, ND, SH] (row-major shard transposed)."""
                dst = wk.tile([P, ND, SH], F32R, tag="HT")
                for qm in reversed(range(NSH)):
                    for dm in range(ND):
                        tp = ps.tile([P, P], F32R, tag="tp")
                        nc.tensor.transpose(tp[:], src[:, qm, ts(dm, P)], ident_r[:])
                        nc.vector.tensor_copy(dst[:, dm, ts(qm, P)], tp[:])
                return dst

            out1T = transpose_rows(out1, ND)

            # ---- block 2 shard projections ----
            k2T = wk.tile([P, NK, SH], BF16, tag="kv_k")
            for m in range(NK):
                pm = ps.tile([P, SH], F32, tag="mm")
                for k in range(ND):
                    nc.tensor.matmul(pm[:], W_r["Wk2"][:, k, ts(m, P)], out1T[:, k, :],
                                     start=(k == 0), stop=(k == ND - 1))
                nc.vector.tensor_copy(k2T[:, m, :], pm[:])
            # gather 1 (k2T) fires while q2T/v2 are still being computed
            gk_in = dram.tile([NK * P * SH], BF16)
            nc.sync.dma_start(
                gk_in[:].rearrange("(m p j) -> p m j", m=NK, p=P), k2T[:])
            gk_out = shd.tile([C, NK * P * SH], BF16, addr_space="Shared")
            nc.gpsimd.collective_compute(
                "AllGather", mybir.AluOpType.bypass,
                replica_groups=[list(range(C))],
                ins=[gk_in.opt()], outs=[gk_out.opt()],
            )
            k2T_full = big.tile([P, NK, S], BF16, tag="kT")
            for m in range(NK):
                nc.sync.dma_start(
                    k2T_full[:, m, :].rearrange("p (c j) -> p c j", c=C),
                    gk_out[:, m * P * SH:(m + 1) * P * SH].rearrange(
                        "c (p j) -> p c j", p=P))
            q2T = wk.tile([P, NK, SH], BF16, tag="qT")
            for m in range(NK):
                pm = ps.tile([P, SH], F32, tag="mm")
                for k in range(ND):
                    nc.tensor.matmul(pm[:], W_r["Wq2"][:, k, ts(m, P)], out1T[:, k, :],
                                     start=(k == 0), stop=(k == ND - 1))
                nc.vector.tensor_scalar_add(q2T[:, m, :], pm[:], bq2_sb[:, m:m + 1])

            def vproj_norm(hT, out_dtype):
                """v = normalize_rows(h @ Wv2 + bv2) for this core's 256 rows."""
                v_sb = wk.tile([P, NSH, D], out_dtype, tag="kv_v")
                for r in range(NSH):
                    pm = ps.tile([P, D], F32, tag="mm")
                    for k in range(ND):
                        nc.tensor.matmul(pm[:], hT[:, k, ts(r, P)], W_r["Wv2"][:, k, :],
                                         start=(k == 0), stop=False)
                    nc.tensor.matmul(pm[:], ones_r[:], bv2_r[:], start=False, stop=True)
                    scr = sm.tile([P, D], F32, tag="scr")
                    ssum = sm.tile([P, 1], F32, tag="ssum")
                    nc.scalar.activation(scr[:], pm[:], AF.Square, accum_out=ssum[:])
                    nrm = sm.tile([P, 1], F32, tag="nrm")
                    nc.scalar.sqrt(nrm[:], ssum[:])
                    rn = sm.tile([P, 1], F32, tag="rn")
                    nc.vector.reciprocal(rn[:], nrm[:])
                    nc.scalar.activation(v_sb[:, r, :], pm[:], AF.Copy, scale=rn[:])
                return v_sb

            v2 = vproj_norm(out1T, BF16)

            gv_in = dram.tile([NSH * P * D], BF16)
            nc.sync.dma_start(
                gv_in[:].rearrange("(r p d) -> p r d", r=NSH, p=P), v2[:])
            gv_out = shd.tile([C, NSH * P * D], BF16, addr_space="Shared")
            nc.gpsimd.collective_compute(
                "AllGather", mybir.AluOpType.bypass,
                replica_groups=[list(range(C))],
                ins=[gv_in.opt()], outs=[gv_out.opt()],
            )
            v2_full = big.tile([P, NS, D], BF16, tag="XV")
            for r in range(NSH):
                off = r * P * D
                nc.sync.dma_start(
                    v2_full[:].rearrange("p (c r) d -> p c r d", c=C)[:, :, r, :],
                    gv_out[:, off:off + P * D].rearrange("c (p d) -> p c d", p=P))

            # ---- block 2 attention ----
            hidden = wk.tile([P, NSH, D], F32R, tag="H")
            attention(q2T, k2T_full, v2_full, hidden, BF16)
            hT = transpose_rows(hidden, ND)

            # broadcast hidden[-1] (core 7's last local row)
            gB_in = dram.tile([D], F32)
            nc.gpsimd.dma_start(gB_in[:].rearrange("(dm p) -> p dm", p=P), hT[:, :, SH - 1])
            gB_out = shd.tile([C, D], F32, addr_space="Shared")
            nc.gpsimd.collective_compute(
                "AllGather", mybir.AluOpType.bypass,
                replica_groups=[list(range(C))],
                ins=[gB_in.opt()], outs=[gB_out.opt()],
            )
            hl_r = sm.tile([P, ND], F32, tag="hl")
            nc.sync.dma_start(hl_r[:], gB_out[C - 1, :].rearrange("(dm p) -> p dm", p=P))

            # ---- block 3 (flash-style partials over this core's 256 keys) ----
            k3T = wk.tile([P, NK, SH], F32, tag="kv_k")
            for m in range(NK):
                pm = ps.tile([P, SH], F32, tag="mm")
                for k in range(ND):
                    nc.tensor.matmul(pm[:], W_r["Wk2"][:, k, ts(m, P)], hT[:, k, :],
                                     start=(k == 0), stop=(k == ND - 1))
                nc.vector.tensor_copy(k3T[:, m, :], pm[:])
            v3 = vproj_norm(hT, F32)

            # q3 = Wq2^T @ h_last + bq2
            q3 = sm.tile([P, NK], F32, tag="q3")
            for fm in range(NK):
                pm = ps.tile([P, 1], F32, tag="mm")
                for dm in range(ND):
                    nc.tensor.matmul(pm[:], Wq2f[:, dm, ts(fm, P)], hl_r[:, dm:dm + 1],
                                     start=(dm == 0), stop=(dm == ND - 1))
                nc.vector.tensor_scalar_add(q3[:, fm:fm + 1], pm[:], bq2_sb[:, fm:fm + 1])

            # s3 (scores for my 256 keys; |s3| <= ~4 so exp needs no max shift)
            s3p = ps.tile([P, NSH], F32, tag="tp")
            for n in range(NSH):
                for fm in range(NK):
                    nc.tensor.matmul(s3p[:, n:n + 1], k3T[:, fm, ts(n, P)], q3[:, fm:fm + 1],
                                     start=(fm == 0), stop=(fm == NK - 1))
            p3 = sm.tile([P, NSH], F32, tag="p3")
            nc.scalar.activation(p3[:], s3p[:], AF.Exp)

            # partial numerator o3 = p3 @ v3 and partial denominator l3 = sum p3
            o3p = ps.tile([1, D], F32, tag="mm")
            for n in range(NSH):
                nc.tensor.matmul(o3p[:], p3[:, n:n + 1], v3[:, n, :],
                                 start=(n == 0), stop=(n == NSH - 1))
            l3p = ps.tile([1, 1], F32, tag="tp")
            for n in range(NSH):
                nc.tensor.matmul(l3p[:], p3[:, n:n + 1], onescol_f[:],
                                 start=(n == 0), stop=(n == NSH - 1))
            ol = wk.tile([1, D + 1], F32, tag="ol")
            nc.vector.tensor_copy(ol[:, 0:D], o3p[:])
            nc.vector.tensor_copy(ol[:, D:D + 1], l3p[:])

            ar_in = dram.tile([1, D + 1], F32)
            nc.sync.dma_start(ar_in[:], ol[:])
            ar_out = shd.tile([C, D + 1], F32, addr_space="Shared")
            nc.gpsimd.collective_compute(
                "AllGather", mybir.AluOpType.bypass,
                replica_groups=[list(range(C))],
                ins=[ar_in.opt()], outs=[ar_out.opt()],
            )
            rb = wk.tile([1, D + 1, C], F32, tag="rb")
            nc.sync.dma_start(rb[:], ar_out[:].rearrange("c (o e) -> o e c", o=1))
            tot = wk.tile([1, D + 1], F32, tag="tot")
            nc.vector.reduce_sum(tot[:], rb[:], axis=mybir.AxisListType.X)
            rl3 = sm.tile([1, 1], F32, tag="rl3")
            nc.vector.reciprocal(rl3[:], tot[:, D:D + 1])
            fin = wk.tile([1, D], F32, tag="fin")
            nc.vector.tensor_scalar_mul(fin[:], tot[:, 0:D], rl3[:])
            nc.sync.dma_start(out_ext[:].rearrange("(a b) -> a b", a=1), fin[:])

    nc.finalize()
    return nc


def kernel(**inputs):
    from concourse.bass_utils import run_bass_kernel_spmd

    f = lambda k: np.ascontiguousarray(np.asarray(inputs[k], dtype=np.float32))
    x0 = f("x")[0]                       # [S, D]; batches 1..7 are dead
    xT = np.ascontiguousarray(x0.T)      # [D, S]
    base = {
        "xT": xT, "x0": x0,
        "Wk1": f("Wk1"), "Wq1": f("Wq1"), "Wk2": f("Wk2"), "Wq2": f("Wq2"),
        "Wv2": f("Wv2"), "bq1": f("bq1"), "bq2": f("bq2"),
        "bv2row": f("bv2").reshape(1, D),
        "ones": np.ones((1, P), np.float32),
        "onescol": np.ones((P, 1), np.float32),
        "ident": np.eye(P, dtype=np.float32),
    }
    in_maps = [
        {**base, "xTq": np.ascontiguousarray(xT[:, c * SH:(c + 1) * SH])}
        for c in range(C)
    ]

    if "nc" not in _cache:
        _cache["nc"] = _build()
    res = run_bass_kernel_spmd(_cache["nc"], in_maps, list(range(C)))
    return res.results[0]["out"].astype(np.float32)


if __name__ == "__main__":
    d = np.load("/root/problem/inputs.npz")
    out = kernel(**{k: d[k] for k in d.files})
    ref = np.load("/root/problem/ref_out.npy")
    rel = np.abs(out - ref).max() / np.abs(ref).max()
    print("Relative error:", rel)



# revision 11
# speedup vs baseline: 2.3637x; 2.3637x over previous
"""Trainium2 Bass kernel for nn_ModelAttention2Layers (B=8, S=2048, D=512, K=256).

Only batch 0 matters (the reference returns final[0, -1, :]), so the 2048-query
sequence of batch 0 is sharded across the 8 cores (256 queries each).

All cross-core data movement uses relative-addressed remote_dma_broadcast
(SBUF -> SBUF, ~3-6us per gather) instead of collective_compute (15us fixed
overhead + 40GB/s each in the perf model):
  - block 2's k2/v2 shards are allgathered slot-by-core-id via a tc.Switch on
    the core id (one 8-dest broadcast per core, self included),
  - hidden[-1] (held by core 7) is broadcast the same way (16B/partition),
  - block 3 runs flash-style on local keys; the [o|l] partials are gathered
    and summed on every core.
Receive-side ordering: a Pool nop "gate" gets a wait on the remote semaphore
injected AFTER tile scheduling (the scheduling sim cannot satisfy
remotely-incremented semaphores), and every reader of a gathered tile gets an
explicit dependency edge on the gate.

Matmuls run in float32r / bf16 (full PE rate); k-projection biases are dropped
(softmax-invariant).
"""
import sys

sys.path.insert(0, "/opt/trn_rl_repo")

import numpy as np

S, D, K, P, C = 2048, 512, 256, 128, 8
SH = S // C          # 256 queries/keys per core
ND, NK, NS, NSH = D // P, K // P, S // P, SH // P   # 4, 2, 16, 2
TRN2_NC_BASE = (0, 1, 2, 3, 6, 7, 4, 5)
RDESTS = [(0, TRN2_NC_BASE[s]) for s in range(C)]   # relative, self included
RSEM_TARGET = C * (16 // C)                          # 8 senders x 2

_cache = {}


def _build():
    import concourse.bass as bass
    import concourse.tile as tile
    from concourse import mybir, bacc
    from bass_rust import add_dep_helper

    F32 = mybir.dt.float32
    F32R = mybir.dt.float32r
    BF16 = mybir.dt.bfloat16
    AF = mybir.ActivationFunctionType
    ts = bass.ts

    nc = bacc.Bacc()

    ins = {}
    for name, shape, dt in [
        ("xT", [D, S], F32), ("x0", [S, D], F32), ("xTq", [D, SH], F32),
        ("Wk1", [D, K], F32), ("Wq1", [D, K], F32), ("Wk2", [D, K], F32),
        ("Wq2", [D, K], F32), ("Wv2", [D, D], F32),
        ("bq1", [K], F32), ("bq2", [K], F32), ("bv2row", [1, D], F32),
        ("ones", [1, P], F32), ("onescol", [P, 1], F32), ("ident", [P, P], F32),
        ("coreid", [1, 1], mybir.dt.int32),
    ]:
        ins[name] = nc.dram_tensor(name, shape, dt, kind="ExternalInput")
    out_ext = nc.dram_tensor("out", [D], F32, kind="ExternalOutput")

    gates = []  # (nop instruction, semaphore, target) -> wait injected post-schedule

    with tile.TileContext(nc) as tc:
        with tc.tile_pool(name="const", bufs=1) as cw, \
             tc.tile_pool(name="big", bufs=1) as big, \
             tc.tile_pool(name="work", bufs=1) as wk, \
             tc.tile_pool(name="send", bufs=1) as snd, \
             tc.tile_pool(name="gath", bufs=1) as gth, \
             tc.tile_pool(name="pp", bufs=2) as pp, \
             tc.tile_pool(name="small", bufs=2) as sm, \
             tc.tile_pool(name="ps", bufs=1, space="PSUM") as ps:

            # ---- semaphores / core id ----
            rsem_k2 = nc.alloc_semaphore("rsem_k2")
            rsem_v2 = nc.alloc_semaphore("rsem_v2")
            rsem_hl = nc.alloc_semaphore("rsem_hl")
            rsem_ol = nc.alloc_semaphore("rsem_ol")
            lsem = nc.alloc_semaphore("lsem")

            cid_sb = cw.tile([1, 1], mybir.dt.int32)
            nc.scalar.dma_start(cid_sb[:], ins["coreid"][:])
            cid_reg = nc.gpsimd.alloc_register("cid")
            nc.gpsimd.reg_load(cid_reg, cid_sb[0:1, 0:1])
            from bass_rust import RuntimeValue
            cid_val = RuntimeValue(cid_reg, min_val=0, max_val=C - 1)

            triggers = []

            def bcast_send(full_tile, src_tile, rsem, name):
                """Switch on core id; core j broadcasts src into slot j of
                full_tile on all 8 cores (self included)."""
                for j in tc.Switch(cid_val, C, hint=f"ag_{name}"):
                    nc.gpsimd.remote_dma_broadcast(
                        full_tile[:, j], src_tile[:],
                        remote_sem=rsem, local_sem=lsem, rdests=RDESTS)
                    triggers.append(nc.gpsimd.trigger_dma(count=None))

            def make_gate(rsem, name):
                """Pool nop that (post-scheduling) waits for all 8 broadcasts.
                Ordered after every trigger emitted so far, so a blocked gate
                never delays a send."""
                gate = nc.gpsimd.nop(nofuse=True, hint=f"gate_{name}")
                for t in triggers:
                    add_dep_helper(gate.ins, t.ins, sync=False,
                                   reason="sends before gate")
                gates.append((gate, rsem, RSEM_TARGET))
                return gate

            # ---- input loads ----
            # x / weights load straight into f32r tiles (same bits) on HWDGE
            # queues; Pool stays free for remote-DMA desc generation.
            xTq_r = cw.tile([P, ND, SH], F32R)
            nc.sync.dma_start(xTq_r[:], ins["xTq"][:].bitcast(F32R).rearrange("(k p) j -> p k j", p=P))
            W_r = {}
            for w, ncol, eng in [("Wk1", K, nc.scalar), ("Wq1", K, nc.scalar),
                                 ("Wk2", K, nc.scalar), ("Wq2", K, nc.scalar),
                                 ("Wv2", D, nc.scalar)]:
                W_r[w] = cw.tile([P, ND, ncol], F32R, name=f"W_{w}", tag=f"W_{w}")
                eng.dma_start(W_r[w][:], ins[w][:].bitcast(F32R).rearrange("(k p) n -> p k n", p=P))
            xT_r = big.tile([P, ND, S], F32R, tag="XT")
            for cb in range(4):
                nc.sync.dma_start(
                    xT_r[:, :, ts(cb, 512)],
                    ins["xT"][:].bitcast(F32R).rearrange("(k p) s -> p k s", p=P)[:, :, ts(cb, 512)])
            x0_r = big.tile([P, NS, D], F32R, tag="XV")
            for cb in range(4):
                nc.sync.dma_start(
                    x0_r[:, 4 * cb:4 * cb + 4, :],
                    ins["x0"][:].bitcast(F32R).rearrange("(n p) d -> p n d", p=P)[:, 4 * cb:4 * cb + 4, :])
            bq1_sb = cw.tile([P, NK], F32)
            nc.scalar.dma_start(bq1_sb[:], ins["bq1"][:].rearrange("(m p) -> p m", p=P))
            bq2_sb = cw.tile([P, NK], F32)
            nc.scalar.dma_start(bq2_sb[:], ins["bq2"][:].rearrange("(m p) -> p m", p=P))
            bv2_r = cw.tile([1, D], F32R)
            nc.scalar.dma_start(bv2_r[:], ins["bv2row"][:].bitcast(F32R))
            ones_r = cw.tile([1, P], F32R)
            nc.scalar.dma_start(ones_r[:], ins["ones"][:].bitcast(F32R))
            onescol_f = cw.tile([P, 1], F32)
            nc.scalar.dma_start(onescol_f[:], ins["onescol"][:])
            ident_r = cw.tile([P, P], F32R)
            nc.scalar.dma_start(ident_r[:], ins["ident"][:].bitcast(F32R))

            # ---- block 1 projections ----
            # k1T full [K, S], computed redundantly (no bias: softmax-invariant)
            k1T = big.tile([P, NK, S], F32R, tag="kT")
            for cb in range(S // 512):
                for m in range(NK):
                    pm = ps.tile([P, 512], F32, tag="mm")
                    for k in range(ND):
                        nc.tensor.matmul(pm[:], W_r["Wk1"][:, k, ts(m, P)],
                                         xT_r[:, k, ts(cb, 512)],
                                         start=(k == 0), stop=(k == ND - 1))
                    nc.vector.tensor_copy(k1T[:, m, ts(cb, 512)], pm[:])
            # q1T shard [K, SH] with bias bq1
            q1T = wk.tile([P, NK, SH], F32R, tag="qT")
            for m in range(NK):
                pm = ps.tile([P, SH], F32, tag="mm")
                for k in range(ND):
                    nc.tensor.matmul(pm[:], W_r["Wq1"][:, k, ts(m, P)], xTq_r[:, k, :],
                                     start=(k == 0), stop=(k == ND - 1))
                nc.vector.tensor_scalar_add(q1T[:, m, :], pm[:], bq1_sb[:, m:m + 1])

            def attention(qT, score_jobs, v_blocks, out_dst, pt_dtype,
                          score_gate=None, av_gate=None):
                """out_dst[:, qm, :] = softmax(q.k^T) @ V for this core's queries.

                score_jobs(qm) yields (sc_slice_len, [(lhsT, rhs)] accumulation
                groups) covering the full 2048 keys in order; v_blocks[i] is the
                [128, D] value AP for key block i (same order)."""
                for qm in range(NSH):
                    sc = ps.tile([P, S], F32, tag="sc")
                    for off, sz, accs in score_jobs(qm):
                        for i, (lhsT, rhs) in enumerate(accs):
                            mm = nc.tensor.matmul(sc[:, off:off + sz], lhsT, rhs,
                                                  start=(i == 0),
                                                  stop=(i == len(accs) - 1))
                            if score_gate is not None:
                                add_dep_helper(mm.ins, score_gate.ins, sync=True,
                                               reason="gathered keys")
                    mx = sm.tile([P, 1], F32, tag="mx")
                    nc.vector.reduce_max(mx[:], sc[:].rearrange("p (a b) -> p a b", a=4),
                                         axis=mybir.AxisListType.XY)
                    nm = sm.tile([P, 1], F32, tag="nm")
                    nc.vector.tensor_scalar_mul(nm[:], mx[:], -1.0)
                    Pt = pp.tile([P, S], F32R, tag="P")
                    lsum = sm.tile([P, 4], F32, tag="lsum")
                    for ks in range(4):
                        nc.scalar.activation(Pt[:, ts(ks, 512)], sc[:, ts(ks, 512)],
                                             AF.Exp, bias=nm[:],
                                             accum_out=lsum[:, ks:ks + 1])
                    l = sm.tile([P, 1], F32, tag="l")
                    nc.vector.reduce_sum(l[:], lsum[:], axis=mybir.AxisListType.X)
                    rl = sm.tile([P, 1], F32, tag="rl")
                    nc.vector.reciprocal(rl[:], l[:])
                    PT = pp.tile([P, NS, P], pt_dtype, tag="PT")
                    for n in range(NS):
                        tp = ps.tile([P, P], F32R, tag="tp")
                        nc.tensor.transpose(tp[:], Pt[:, ts(n, P)], ident_r[:])
                        nc.vector.tensor_copy(PT[:, n, :], tp[:])
                    av = ps.tile([P, D], F32, tag="mm")
                    for n in range(NS):
                        mm = nc.tensor.matmul(av[:], PT[:, n, :], v_blocks[n],
                                              start=(n == 0), stop=(n == NS - 1))
                        if av_gate is not None:
                            add_dep_helper(mm.ins, av_gate.ins, sync=True,
                                           reason="gathered values")
                    nc.scalar.activation(out_dst[:, qm, :], av[:], AF.Copy, scale=rl[:])

            def jobs1(qm):
                for ks in range(4):
                    yield ks * 512, 512, [
                        (q1T[:, dm, ts(qm, P)], k1T[:, dm, ts(ks, 512)])
                        for dm in range(NK)]

            out1 = wk.tile([P, NSH, D], F32R, tag="H")
            attention(q1T, jobs1, [x0_r[:, n, :] for n in range(NS)], out1, F32R)

            def transpose_rows(src, hl_out=None):
                """src [P, NSH, D] -> dst [P, ND, SH]; optionally extract the
                last row (query SH-1) into hl_out [P, ND]."""
                dst = wk.tile([P, ND, SH], F32R, tag="HT")
                for qm in reversed(range(NSH)):
                    for dm in range(ND):
                        tp = ps.tile([P, P], F32R, tag="tp")
                        nc.tensor.transpose(tp[:], src[:, qm, ts(dm, P)], ident_r[:])
                        nc.vector.tensor_copy(dst[:, dm, ts(qm, P)], tp[:])
                        if hl_out is not None and qm == NSH - 1:
                            nc.vector.tensor_copy(hl_out[:, dm:dm + 1],
                                                  tp[:, P - 1:P])
                return dst

            out1T = transpose_rows(out1)

            # ---- block 2 shard projections + gathers ----
            k2T = snd.tile([P, NK, SH], BF16, tag="snd_k")
            for m in range(NK):
                pm = ps.tile([P, SH], F32, tag="mm")
                for k in range(ND):
                    nc.tensor.matmul(pm[:], W_r["Wk2"][:, k, ts(m, P)], out1T[:, k, :],
                                     start=(k == 0), stop=(k == ND - 1))
                nc.vector.tensor_copy(k2T[:, m, :], pm[:])
            k2T_full = gth.tile([P, C, NK, SH], BF16, tag="g_k2")
            bcast_send(k2T_full, k2T, rsem_k2, "k2")

            q2T = wk.tile([P, NK, SH], BF16, tag="qT2")
            for m in range(NK):
                pm = ps.tile([P, SH], F32, tag="mm")
                for k in range(ND):
                    nc.tensor.matmul(pm[:], W_r["Wq2"][:, k, ts(m, P)], out1T[:, k, :],
                                     start=(k == 0), stop=(k == ND - 1))
                nc.vector.tensor_scalar_add(q2T[:, m, :], pm[:], bq2_sb[:, m:m + 1])

            def vproj_norm(hT, out_dtype, pool, tag):
                """v = normalize_rows(h @ Wv2 + bv2) for this core's 256 rows."""
                v_sb = pool.tile([P, NSH, D], out_dtype, tag=tag)
                for r in range(NSH):
                    pm = ps.tile([P, D], F32, tag="mm")
                    for k in range(ND):
                        nc.tensor.matmul(pm[:], hT[:, k, ts(r, P)], W_r["Wv2"][:, k, :],
                                         start=(k == 0), stop=False)
                    nc.tensor.matmul(pm[:], ones_r[:], bv2_r[:], start=False, stop=True)
                    scr = sm.tile([P, D], F32, tag="scr")
                    ssum = sm.tile([P, 1], F32, tag="ssum")
                    nc.scalar.activation(scr[:], pm[:], AF.Square, accum_out=ssum[:])
                    nrm = sm.tile([P, 1], F32, tag="nrm")
                    nc.scalar.sqrt(nrm[:], ssum[:])
                    rn = sm.tile([P, 1], F32, tag="rn")
                    nc.vector.reciprocal(rn[:], nrm[:])
                    nc.scalar.activation(v_sb[:, r, :], pm[:], AF.Copy, scale=rn[:])
                return v_sb

            v2 = vproj_norm(out1T, BF16, snd, "snd_v")
            v2_full = gth.tile([P, C, NSH, D], BF16, tag="g_v2")
            bcast_send(v2_full, v2, rsem_v2, "v2")
            gate_k2 = make_gate(rsem_k2, "k2")
            gate_v2 = make_gate(rsem_v2, "v2")

            # ---- block 2 attention (keys in slot-major order) ----
            def jobs2(qm):
                for j in range(C):
                    yield j * SH, SH, [
                        (q2T[:, dm, ts(qm, P)], k2T_full[:, j, dm, :])
                        for dm in range(NK)]

            v2_blocks = [v2_full[:, n // NSH, n % NSH, :] for n in range(NS)]
            hidden = wk.tile([P, NSH, D], F32R, tag="H")
            attention(q2T, jobs2, v2_blocks, hidden, BF16,
                      score_gate=gate_k2, av_gate=gate_v2)

            hl_c = snd.tile([P, ND], F32R, tag="snd_hl")
            hT = transpose_rows(hidden, hl_out=hl_c)

            hlg = gth.tile([P, C, ND], F32R, tag="g_hl")
            bcast_send(hlg, hl_c, rsem_hl, "hl")
            gate_hl = make_gate(rsem_hl, "hl")

            # ---- block 3 (flash-style partials over this core's 256 keys) ----
            k3T = wk.tile([P, NK, SH], F32, tag="k3")
            for m in range(NK):
                pm = ps.tile([P, SH], F32, tag="mm")
                for k in range(ND):
                    nc.tensor.matmul(pm[:], W_r["Wk2"][:, k, ts(m, P)], hT[:, k, :],
                                     start=(k == 0), stop=(k == ND - 1))
                nc.vector.tensor_copy(k3T[:, m, :], pm[:])
            v3 = vproj_norm(hT, F32, wk, "v3")

            # q3 = Wq2^T @ hidden[-1] + bq2 ; hidden[-1] is core 7's slot
            q3 = sm.tile([P, NK], F32, tag="q3")
            for fm in range(NK):
                pm = ps.tile([P, 1], F32, tag="mm")
                for dm in range(ND):
                    mm = nc.tensor.matmul(pm[:], W_r["Wq2"][:, dm, ts(fm, P)],
                                          hlg[:, C - 1, dm:dm + 1],
                                          start=(dm == 0), stop=(dm == ND - 1))
                    add_dep_helper(mm.ins, gate_hl.ins, sync=True, reason="hl gather")
                nc.vector.tensor_scalar_add(q3[:, fm:fm + 1], pm[:], bq2_sb[:, fm:fm + 1])

            # s3 (scores for my 256 keys; |s3| small so exp needs no max shift)
            s3p = ps.tile([P, NSH], F32, tag="tp")
            for n in range(NSH):
                for fm in range(NK):
                    nc.tensor.matmul(s3p[:, n:n + 1], k3T[:, fm, ts(n, P)],
                                     q3[:, fm:fm + 1],
                                     start=(fm == 0), stop=(fm == NK - 1))
            p3 = sm.tile([P, NSH], F32, tag="p3")
            nc.scalar.activation(p3[:], s3p[:], AF.Exp)

            # partial numerator oT [128,4] (d on partitions) + replicated l
            ol_ps = ps.tile([P, ND + 1], F32, tag="mm")
            for dm in range(ND):
                for n in range(NSH):
                    nc.tensor.matmul(ol_ps[:, dm:dm + 1], v3[:, n, ts(dm, P)],
                                     p3[:, n:n + 1],
                                     start=(n == 0), stop=(n == NSH - 1))
            l3p = ps.tile([1, 1], F32, tag="tp")
            for n in range(NSH):
                nc.tensor.matmul(l3p[:], p3[:, n:n + 1], onescol_f[:],
                                 start=(n == 0), stop=(n == NSH - 1))
            l3f = sm.tile([1, 1], F32R, tag="l3f")
            nc.vector.tensor_copy(l3f[:], l3p[:])
            nc.tensor.matmul(ol_ps[:, ND:ND + 1], ones_r[:], l3f[:],
                             start=True, stop=True)
            ol = snd.tile([P, ND + 1], F32, tag="snd_ol")
            nc.vector.tensor_copy(ol[:], ol_ps[:])

            olg = gth.tile([P, C, ND + 1], F32, tag="g_ol")
            bcast_send(olg, ol, rsem_ol, "ol")
            gate_ol = make_gate(rsem_ol, "ol")

            tot = wk.tile([P, ND + 1], F32, tag="tot")
            rs = nc.vector.reduce_sum(tot[:], olg[:].rearrange("p c e -> p e c"),
                                      axis=mybir.AxisListType.X)
            add_dep_helper(rs.ins, gate_ol.ins, sync=True, reason="ol gather")
            rl3 = sm.tile([P, 1], F32, tag="rl3")
            nc.vector.reciprocal(rl3[:], tot[:, ND:ND + 1])
            fin = wk.tile([P, ND], F32, tag="fin")
            nc.vector.tensor_scalar_mul(fin[:], tot[:, 0:ND], rl3[:])
            nc.sync.dma_start(out_ext[:].rearrange("(k p) -> p k", p=P), fin[:])

    for gate, sem, target in gates:
        gate.wait_op(sem, target, "sem-ge")
    nc.finalize()
    return nc


def kernel(**inputs):
    from concourse.bass_utils import run_bass_kernel_spmd

    f = lambda k: np.ascontiguousarray(np.asarray(inputs[k], dtype=np.float32))
    x0 = f("x")[0]                       # [S, D]; batches 1..7 are dead
    xT = np.ascontiguousarray(x0.T)      # [D, S]
    base = {
        "xT": xT, "x0": x0,
        "Wk1": f("Wk1"), "Wq1": f("Wq1"), "Wk2": f("Wk2"), "Wq2": f("Wq2"),
        "Wv2": f("Wv2"), "bq1": f("bq1"), "bq2": f("bq2"),
        "bv2row": f("bv2").reshape(1, D),
        "ones": np.ones((1, P), np.float32),
        "onescol": np.ones((P, 1), np.float32),
        "ident": np.eye(P, dtype=np.float32),
    }
    in_maps = [
        {**base,
         "xTq": np.ascontiguousarray(xT[:, c * SH:(c + 1) * SH]),
         "coreid": np.array([[c]], np.int32)}
        for c in range(C)
    ]

    if "nc" not in _cache:
        _cache["nc"] = _build()
    res = run_bass_kernel_spmd(_cache["nc"], in_maps, list(range(C)))
    return res.results[0]["out"].astype(np.float32)


if __name__ == "__main__":
    d = np.load("/root/problem/inputs.npz")
    out = kernel(**{k: d[k] for k in d.files})
    ref = np.load("/root/problem/ref_out.npy")
    rel = np.abs(out - ref).max() / np.abs(ref).max()
    print("Relative error:", rel)


# revision 12
# speedup vs baseline: 2.6115x; 1.1048x over previous
"""Trainium2 Bass kernel for nn_ModelAttention2Layers (B=8, S=2048, D=512, K=256).

Only batch 0 matters (the reference returns final[0, -1, :]), so the 2048-query
sequence of batch 0 is sharded across the 8 cores (256 queries each).

All cross-core data movement uses relative-addressed remote_dma_broadcast
(SBUF -> SBUF) instead of collective_compute (15us fixed overhead + 40GB/s
each in the perf model). Each allgather is a tc.Switch on the core id: core j
issues one 8-destination broadcast (self included) whose out slot is j, so
slot j always holds core j's shard (keys stay in linear order):
  - block 1's k1 shards (k1 = Wk1^T x is sharded, not recomputed 8x; this
    also removes the 4MB full-xT load),
  - block 2's k2 and v2 shards,
  - hidden[-1] (core 7's last row; 16B/partition),
  - block 3's flash-style [o|l] partials, summed on every core.
Receive-side ordering: a Pool nop "gate" gets a wait on the remote semaphore
injected AFTER tile scheduling (the scheduling sim cannot satisfy
remotely-incremented semaphores), and every reader of a gathered tile gets an
explicit dependency edge on the gate.

Activation-table discipline: only {Exp, Ln, Square, Copy} are used (one
act-func table -> no 1.3us table reloads); 1/sqrt(s) = exp(-0.5*ln(s)).
Matmuls run in float32r / bf16 (full PE rate); k-projection biases are dropped
(softmax-invariant; they are zero in setup_inputs anyway).
"""
import sys

sys.path.insert(0, "/opt/trn_rl_repo")

import numpy as np

S, D, K, P, C = 2048, 512, 256, 128, 8
SH = S // C          # 256 queries/keys per core
ND, NK, NS, NSH = D // P, K // P, S // P, SH // P   # 4, 2, 16, 2
TRN2_NC_BASE = (0, 1, 2, 3, 6, 7, 4, 5)
RDESTS = [(0, TRN2_NC_BASE[s]) for s in range(C)]   # relative, self included
RSEM_TARGET = C * (16 // C)                          # 8 senders x 2

# packed-constants layout (one [P, PACKW] f32 DMA): see _pack() below
COL_BQ1, COL_BQ2 = 0, NK
COL_ONESCOL = 2 * NK
COL_NEGHALF = 2 * NK + 1
COL_CID = 2 * NK + 2
COL_IDENT = 2 * NK + 3
COL_BV2 = COL_IDENT + P          # row 0 only
COL_ONESROW = COL_BV2 + D        # row 0 only
PACKW = COL_ONESROW + P

_cache = {}


def _build():
    import concourse.bass as bass
    import concourse.tile as tile
    from concourse import mybir, bacc
    from bass_rust import add_dep_helper, RuntimeValue

    F32 = mybir.dt.float32
    F32R = mybir.dt.float32r
    BF16 = mybir.dt.bfloat16
    I32 = mybir.dt.int32
    AF = mybir.ActivationFunctionType
    ts = bass.ts

    nc = bacc.Bacc()

    ins = {}
    for name, shape, dt in [
        ("x0", [S, D], F32), ("xTq", [D, SH], F32),
        ("Wk1", [D, K], F32), ("Wq1", [D, K], F32), ("Wk2", [D, K], F32),
        ("Wq2", [D, K], F32), ("Wv2", [D, D], F32),
        ("pack", [P, PACKW], F32),
    ]:
        ins[name] = nc.dram_tensor(name, shape, dt, kind="ExternalInput")
    out_ext = nc.dram_tensor("out", [D], F32, kind="ExternalOutput")

    gates = []  # (nop instruction, semaphore, target) -> wait injected post-schedule

    with tile.TileContext(nc) as tc:
        with tc.tile_pool(name="const", bufs=1) as cw, \
             tc.tile_pool(name="big", bufs=1) as big, \
             tc.tile_pool(name="work", bufs=1) as wk, \
             tc.tile_pool(name="send", bufs=1) as snd, \
             tc.tile_pool(name="gath", bufs=1) as gth, \
             tc.tile_pool(name="pp", bufs=2) as pp, \
             tc.tile_pool(name="small", bufs=2) as sm, \
             tc.tile_pool(name="ps", bufs=1, space="PSUM") as ps:

            rsem_k1 = nc.alloc_semaphore("rsem_k1")
            rsem_k2 = nc.alloc_semaphore("rsem_k2")
            rsem_v2 = nc.alloc_semaphore("rsem_v2")
            rsem_hl = nc.alloc_semaphore("rsem_hl")
            rsem_ol = nc.alloc_semaphore("rsem_ol")
            lsem = nc.alloc_semaphore("lsem")

            # ---- input loads ----
            # f32r tiles are loaded with a bitcast (same bits); SP carries
            # xTq + x0, Act carries the weights + the packed constants.
            # Pool stays free for remote-DMA desc generation.
            xTq_r = cw.tile([P, ND, SH], F32R)
            nc.sync.dma_start(xTq_r[:],
                              ins["xTq"][:].bitcast(F32R).rearrange("(k p) j -> p k j", p=P))
            x0_r = big.tile([P, NS, D], F32R, tag="XV")
            for cb in range(4):
                nc.sync.dma_start(
                    x0_r[:, 4 * cb:4 * cb + 4, :],
                    ins["x0"][:].bitcast(F32R).rearrange("(n p) d -> p n d", p=P)[:, 4 * cb:4 * cb + 4, :])
            W_r = {}
            for w, ncol in [("Wk1", K), ("Wq1", K), ("Wk2", K), ("Wq2", K),
                            ("Wv2", D)]:
                W_r[w] = cw.tile([P, ND, ncol], F32R, name=f"W_{w}", tag=f"W_{w}")
                nc.scalar.dma_start(
                    W_r[w][:], ins[w][:].bitcast(F32R).rearrange("(k p) n -> p k n", p=P))
            pk = cw.tile([P, PACKW], F32)
            nc.scalar.dma_start(pk[:], ins["pack"][:])
            bq1_sb = pk[:, COL_BQ1:COL_BQ1 + NK]
            bq2_sb = pk[:, COL_BQ2:COL_BQ2 + NK]
            onescol_f = pk[:, COL_ONESCOL:COL_ONESCOL + 1]
            neghalf = pk[:, COL_NEGHALF:COL_NEGHALF + 1]
            ident_r = pk[:, COL_IDENT:COL_IDENT + P].bitcast(F32R)
            bv2_r = pk[0:1, COL_BV2:COL_BV2 + D].bitcast(F32R)
            ones_r = pk[0:1, COL_ONESROW:COL_ONESROW + P].bitcast(F32R)

            cid_reg = nc.gpsimd.alloc_register("cid")
            nc.gpsimd.reg_load(cid_reg, pk[0:1, COL_CID:COL_CID + 1].bitcast(I32))
            cid_val = RuntimeValue(cid_reg, min_val=0, max_val=C - 1)

            triggers = []

            def bcast_send(full_tile, src_tile, rsem, name):
                """Switch on core id; core j broadcasts src into slot j of
                full_tile on all 8 cores (self included)."""
                for j in tc.Switch(cid_val, C, hint=f"ag_{name}"):
                    nc.gpsimd.remote_dma_broadcast(
                        full_tile[:, j], src_tile[:],
                        remote_sem=rsem, local_sem=lsem, rdests=RDESTS)
                    triggers.append(nc.gpsimd.trigger_dma(count=None))

            def make_gate(rsem, name):
                """Pool nop that (post-scheduling) waits for all 8 broadcasts.
                Ordered after every trigger emitted so far, so a blocked gate
                never delays a send."""
                gate = nc.gpsimd.nop(nofuse=True, hint=f"gate_{name}")
                for t in triggers:
                    add_dep_helper(gate.ins, t.ins, sync=False,
                                   reason="sends before gate")
                gates.append((gate, rsem, RSEM_TARGET))
                return gate

            # ---- block 1 projections (sharded k1) ----
            k1s = snd.tile([P, NK, SH], BF16, tag="snd_k1")
            for m in range(NK):
                pm = ps.tile([P, SH], F32, tag="mm")
                for k in range(ND):
                    nc.tensor.matmul(pm[:], W_r["Wk1"][:, k, ts(m, P)], xTq_r[:, k, :],
                                     start=(k == 0), stop=(k == ND - 1))
                nc.vector.tensor_copy(k1s[:, m, :], pm[:])
            k1_full = gth.tile([P, C, NK, SH], BF16, tag="g_k1")
            bcast_send(k1_full, k1s, rsem_k1, "k1")
            gate_k1 = make_gate(rsem_k1, "k1")

            q1T = wk.tile([P, NK, SH], BF16, tag="qT1")
            for m in range(NK):
                pm = ps.tile([P, SH], F32, tag="mm")
                for k in range(ND):
                    nc.tensor.matmul(pm[:], W_r["Wq1"][:, k, ts(m, P)], xTq_r[:, k, :],
                                     start=(k == 0), stop=(k == ND - 1))
                nc.vector.tensor_scalar_add(q1T[:, m, :], pm[:], bq1_sb[:, m:m + 1])

            def attention(qT, kfull, v_blocks, out_dst, pt_dtype,
                          score_gate=None, av_gate=None):
                """out_dst[:, qm, :] = softmax(q.k^T) @ V for this core's
                queries. Keys are in slot-major (= linear) order."""
                for qm in range(NSH):
                    sc = ps.tile([P, S], F32, tag="sc")
                    mx4 = sm.tile([P, 4], F32, tag="mx4")
                    for ks in range(4):
                        for jj in range(2):
                            j = 2 * ks + jj
                            for dm in range(NK):
                                mm = nc.tensor.matmul(
                                    sc[:, j * SH:(j + 1) * SH],
                                    qT[:, dm, ts(qm, P)], kfull[:, j, dm, :],
                                    start=(dm == 0), stop=(dm == NK - 1))
                                if score_gate is not None:
                                    add_dep_helper(mm.ins, score_gate.ins,
                                                   sync=True, reason="gathered keys")
                        nc.vector.reduce_max(mx4[:, ks:ks + 1], sc[:, ts(ks, 512)],
                                             axis=mybir.AxisListType.X)
                    mx = sm.tile([P, 1], F32, tag="mx")
                    nc.vector.reduce_max(mx[:], mx4[:], axis=mybir.AxisListType.X)
                    nm = sm.tile([P, 1], F32, tag="nm")
                    nc.vector.tensor_scalar_mul(nm[:], mx[:], -1.0)
                    Pt = pp.tile([P, S], F32R, tag="P")
                    lsum = sm.tile([P, 4], F32, tag="lsum")
                    for ks in range(4):
                        nc.scalar.activation(Pt[:, ts(ks, 512)], sc[:, ts(ks, 512)],
                                             AF.Exp, bias=nm[:],
                                             accum_out=lsum[:, ks:ks + 1])
                    l = sm.tile([P, 1], F32, tag="l")
                    nc.vector.reduce_sum(l[:], lsum[:], axis=mybir.AxisListType.X)
                    rl = sm.tile([P, 1], F32, tag="rl")
                    nc.vector.reciprocal(rl[:], l[:])
                    PT = pp.tile([P, NS, P], pt_dtype, tag="PT")
                    for g in range(4):
                        tp = ps.tile([P, 4, P], F32R, tag="tp")
                        for u in range(4):
                            nc.tensor.transpose(tp[:, u, :], Pt[:, ts(4 * g + u, P)],
                                                ident_r)
                        nc.vector.tensor_copy(
                            PT[:, 4 * g:4 * g + 4, :].rearrange("p a b -> p (a b)"),
                            tp[:].rearrange("p a b -> p (a b)"))
                    av = ps.tile([P, D], F32, tag="mm")
                    for n in range(NS):
                        mm = nc.tensor.matmul(av[:], PT[:, n, :], v_blocks[n],
                                              start=(n == 0), stop=(n == NS - 1))
                        if av_gate is not None:
                            add_dep_helper(mm.ins, av_gate.ins, sync=True,
                                           reason="gathered values")
                    nc.scalar.activation(out_dst[:, qm, :], av[:], AF.Copy, scale=rl[:])

            out1 = wk.tile([P, NSH, D], F32R, tag="H")
            attention(q1T, k1_full, [x0_r[:, n, :] for n in range(NS)], out1, F32R,
                      score_gate=gate_k1)

            def transpose_rows(src, hl_out=None):
                """src [P, NSH, D] -> dst [P, ND, SH]; optionally extract the
                last row (query SH-1) into hl_out [P, ND]."""
                dst = wk.tile([P, ND, SH], F32R, tag="HT")
                for qm in reversed(range(NSH)):
                    tp = ps.tile([P, 4, P], F32R, tag="tp")
                    for dm in range(ND):
                        nc.tensor.transpose(tp[:, dm, :], src[:, qm, ts(dm, P)],
                                            ident_r)
                    for dm in range(ND):
                        nc.vector.tensor_copy(dst[:, dm, ts(qm, P)], tp[:, dm, :])
                        if hl_out is not None and qm == NSH - 1:
                            nc.vector.tensor_copy(hl_out[:, dm:dm + 1],
                                                  tp[:, dm, P - 1:P])
                return dst

            out1T = transpose_rows(out1)

            # ---- block 2 shard projections + gathers ----
            k2T = snd.tile([P, NK, SH], BF16, tag="snd_k2")
            for m in range(NK):
                pm = ps.tile([P, SH], F32, tag="mm")
                for k in range(ND):
                    nc.tensor.matmul(pm[:], W_r["Wk2"][:, k, ts(m, P)], out1T[:, k, :],
                                     start=(k == 0), stop=(k == ND - 1))
                nc.vector.tensor_copy(k2T[:, m, :], pm[:])
            k2_full = gth.tile([P, C, NK, SH], BF16, tag="g_k2")
            bcast_send(k2_full, k2T, rsem_k2, "k2")

            q2T = wk.tile([P, NK, SH], BF16, tag="qT2")
            for m in range(NK):
                pm = ps.tile([P, SH], F32, tag="mm")
                for k in range(ND):
                    nc.tensor.matmul(pm[:], W_r["Wq2"][:, k, ts(m, P)], out1T[:, k, :],
                                     start=(k == 0), stop=(k == ND - 1))
                nc.vector.tensor_scalar_add(q2T[:, m, :], pm[:], bq2_sb[:, m:m + 1])

            def rsqrt_act(dstap, srcap):
                """1/sqrt(s) via exp(-0.5*ln(s)) — stays in one act table."""
                t = sm.tile([P, 1], F32, tag="lnt")
                nc.scalar.activation(t[:], srcap, AF.Ln)
                nc.scalar.activation(dstap, t[:], AF.Exp, scale=neghalf)

            def vproj(hT, out_dtype, pool, tag, normalize):
                """v = h @ Wv2 + bv2 for this core's 256 rows; if normalize,
                rows are L2-normalized, else the raw rows and the 1/|row|
                factors (rn [P, NSH]) are returned separately."""
                v_sb = pool.tile([P, NSH, D], out_dtype, tag=tag)
                rn_t = None if normalize else sm.tile([P, NSH], F32, tag="rn3")
                for r in range(NSH):
                    pm = ps.tile([P, D], F32, tag="mm")
                    for k in range(ND):
                        nc.tensor.matmul(pm[:], hT[:, k, ts(r, P)], W_r["Wv2"][:, k, :],
                                         start=(k == 0), stop=False)
                    nc.tensor.matmul(pm[:], ones_r, bv2_r, start=False, stop=True)
                    scr = sm.tile([P, D], F32, tag="scr")
                    ssum = sm.tile([P, 1], F32, tag="ssum")
                    nc.scalar.activation(scr[:], pm[:], AF.Square, accum_out=ssum[:])
                    if normalize:
                        rn = sm.tile([P, 1], F32, tag="rn")
                        rsqrt_act(rn[:], ssum[:])
                        nc.scalar.activation(v_sb[:, r, :], pm[:], AF.Copy, scale=rn[:])
                    else:
                        rsqrt_act(rn_t[:, r:r + 1], ssum[:])
                        nc.vector.tensor_copy(v_sb[:, r, :], pm[:])
                return v_sb, rn_t

            v2, _ = vproj(out1T, BF16, snd, "snd_v2", normalize=True)
            v2_full = gth.tile([P, C, NSH, D], BF16, tag="g_v2")
            bcast_send(v2_full, v2, rsem_v2, "v2")
            gate_k2 = make_gate(rsem_k2, "k2")
            gate_v2 = make_gate(rsem_v2, "v2")

            # ---- block 2 attention ----
            v2_blocks = [v2_full[:, n // NSH, n % NSH, :] for n in range(NS)]
            hidden = wk.tile([P, NSH, D], F32R, tag="H")
            attention(q2T, k2_full, v2_blocks, hidden, BF16,
                      score_gate=gate_k2, av_gate=gate_v2)

            hl_c = snd.tile([P, ND], F32R, tag="snd_hl")
            hT = transpose_rows(hidden, hl_out=hl_c)

            hlg = gth.tile([P, C, ND], F32R, tag="g_hl")
            bcast_send(hlg, hl_c, rsem_hl, "hl")
            gate_hl = make_gate(rsem_hl, "hl")

            # ---- block 3 (flash-style partials over this core's 256 keys).
            # k3/v3/rn3 only need local data and overlap the hl exchange; the
            # 1/|v| factors are folded into p3 so nothing heavy sits on the
            # post-hl critical path.
            k3T = wk.tile([P, NK, SH], F32R, tag="k3")
            for m in range(NK):
                pm = ps.tile([P, SH], F32, tag="mm")
                for k in range(ND):
                    nc.tensor.matmul(pm[:], W_r["Wk2"][:, k, ts(m, P)], hT[:, k, :],
                                     start=(k == 0), stop=(k == ND - 1))
                nc.vector.tensor_copy(k3T[:, m, :], pm[:])
            v3, rn3 = vproj(hT, F32R, wk, "v3", normalize=False)

            # q3 = Wq2^T @ hidden[-1] + bq2 ; hidden[-1] is core 7's slot
            q3 = sm.tile([P, NK], F32R, tag="q3")
            for fm in range(NK):
                pm = ps.tile([P, 1], F32, tag="mm")
                for dm in range(ND):
                    mm = nc.tensor.matmul(pm[:], W_r["Wq2"][:, dm, ts(fm, P)],
                                          hlg[:, C - 1, dm:dm + 1],
                                          start=(dm == 0), stop=(dm == ND - 1))
                    add_dep_helper(mm.ins, gate_hl.ins, sync=True, reason="hl gather")
                nc.vector.tensor_scalar_add(q3[:, fm:fm + 1], pm[:], bq2_sb[:, fm:fm + 1])

            # s3 (scores for my 256 keys; |s3| small so exp needs no max shift)
            s3p = ps.tile([P, NSH], F32, tag="tp")
            for n in range(NSH):
                for fm in range(NK):
                    nc.tensor.matmul(s3p[:, n:n + 1], k3T[:, fm, ts(n, P)],
                                     q3[:, fm:fm + 1],
                                     start=(fm == 0), stop=(fm == NK - 1))
            p3e = sm.tile([P, NSH], F32, tag="p3e")
            nc.scalar.activation(p3e[:], s3p[:], AF.Exp)
            p3 = sm.tile([P, NSH], F32R, tag="p3")
            nc.vector.tensor_tensor(p3[:], p3e[:], rn3[:], mybir.AluOpType.mult)

            # partial numerator oT [128,4] (d on partitions) + replicated l
            ol_ps = ps.tile([P, ND + 1], F32, tag="mm")
            for dm in range(ND):
                for n in range(NSH):
                    nc.tensor.matmul(ol_ps[:, dm:dm + 1], v3[:, n, ts(dm, P)],
                                     p3[:, n:n + 1],
                                     start=(n == 0), stop=(n == NSH - 1))
            l3p = ps.tile([1, 1], F32, tag="tp")
            onescol_r = onescol_f.bitcast(F32R)
            for n in range(NSH):
                nc.tensor.matmul(l3p[:], p3[:, n:n + 1], onescol_r,
                                 start=(n == 0), stop=(n == NSH - 1))
            l3f = sm.tile([1, 1], F32R, tag="l3f")
            nc.vector.tensor_copy(l3f[:], l3p[:])
            nc.tensor.matmul(ol_ps[:, ND:ND + 1], ones_r, l3f[:],
                             start=True, stop=True)
            ol = snd.tile([P, ND + 1], F32, tag="snd_ol")
            nc.vector.tensor_copy(ol[:], ol_ps[:])

            olg = gth.tile([P, C, ND + 1], F32, tag="g_ol")
            bcast_send(olg, ol, rsem_ol, "ol")
            gate_ol = make_gate(rsem_ol, "ol")

            tot = wk.tile([P, ND + 1], F32, tag="tot")
            rs = nc.vector.reduce_sum(tot[:], olg[:].rearrange("p c e -> p e c"),
                                      axis=mybir.AxisListType.X)
            add_dep_helper(rs.ins, gate_ol.ins, sync=True, reason="ol gather")
            rl3 = sm.tile([P, 1], F32, tag="rl3")
            nc.vector.reciprocal(rl3[:], tot[:, ND:ND + 1])
            fin = wk.tile([P, ND], F32, tag="fin")
            nc.vector.tensor_scalar_mul(fin[:], tot[:, 0:ND], rl3[:])
            nc.sync.dma_start(out_ext[:].rearrange("(k p) -> p k", p=P), fin[:])

    for gate, sem, target in gates:
        gate.wait_op(sem, target, "sem-ge")
    nc.finalize()
    return nc


def _pack(c, f):
    pk = np.zeros((P, PACKW), np.float32)
    pk[:, COL_BQ1:COL_BQ1 + NK] = f("bq1").reshape(NK, P).T
    pk[:, COL_BQ2:COL_BQ2 + NK] = f("bq2").reshape(NK, P).T
    pk[:, COL_ONESCOL] = 1.0
    pk[:, COL_NEGHALF] = -0.5
    pk[:, COL_CID] = np.array([c], np.int32).view(np.float32)[0]
    pk[:, COL_IDENT:COL_IDENT + P] = np.eye(P, dtype=np.float32)
    pk[0, COL_BV2:COL_BV2 + D] = f("bv2")
    pk[0, COL_ONESROW:COL_ONESROW + P] = 1.0
    return pk


def kernel(**inputs):
    from concourse.bass_utils import run_bass_kernel_spmd

    f = lambda k: np.ascontiguousarray(np.asarray(inputs[k], dtype=np.float32))
    x0 = f("x")[0]                       # [S, D]; batches 1..7 are dead
    xT = np.ascontiguousarray(x0.T)      # [D, S]
    base = {
        "x0": x0,
        "Wk1": f("Wk1"), "Wq1": f("Wq1"), "Wk2": f("Wk2"), "Wq2": f("Wq2"),
        "Wv2": f("Wv2"),
    }
    in_maps = [
        {**base,
         "xTq": np.ascontiguousarray(xT[:, c * SH:(c + 1) * SH]),
         "pack": _pack(c, f)}
        for c in range(C)
    ]

    if "nc" not in _cache:
        _cache["nc"] = _build()
    res = run_bass_kernel_spmd(_cache["nc"], in_maps, list(range(C)))
    return res.results[0]["out"].astype(np.float32)


if __name__ == "__main__":
    d = np.load("/root/problem/inputs.npz")
    out = kernel(**{k: d[k] for k in d.files})
    ref = np.load("/root/problem/ref_out.npy")
    rel = np.abs(out - ref).max() / np.abs(ref).max()
    print("Relative error:", rel)


# revision 15
# speedup vs baseline: 2.6167x; 1.0020x over previous
"""Trainium2 Bass kernel for nn_ModelAttention2Layers (B=8, S=2048, D=512, K=256).

Only batch 0 matters (the reference returns final[0, -1, :]), so the 2048-query
sequence of batch 0 is sharded across the 8 cores (256 queries each).

All cross-core data movement uses relative-addressed remote_dma_broadcast
(SBUF -> SBUF) instead of collective_compute (15us fixed overhead + 40GB/s
each in the perf model). Each allgather is a tc.Switch on the core id: core j
issues one 8-destination broadcast (self included) whose out slot is j, so
slot j always holds core j's shard (keys stay in linear order):
  - block 1's k1 shards (k1 = Wk1^T x is sharded, not recomputed 8x; this
    also removes the 4MB full-xT load),
  - block 2's k2 and v2 shards,
  - hidden[-1] (core 7's last row; 16B/partition),
  - block 3's flash-style [o|l] partials, summed on every core.
Receive-side ordering: a Pool nop "gate" gets a wait on the remote semaphore
injected AFTER tile scheduling (the scheduling sim cannot satisfy
remotely-incremented semaphores), and every reader of a gathered tile gets an
explicit dependency edge on the gate.

Activation-table discipline: only {Exp, Ln, Square, Copy} are used (one
act-func table -> no 1.3us table reloads); 1/sqrt(s) = exp(-0.5*ln(s)).
Matmuls run in float32r / bf16 (full PE rate); k-projection biases are dropped
(softmax-invariant; they are zero in setup_inputs anyway).
"""
import sys

sys.path.insert(0, "/opt/trn_rl_repo")

import numpy as np

S, D, K, P, C = 2048, 512, 256, 128, 8
SH = S // C          # 256 queries/keys per core
ND, NK, NS, NSH = D // P, K // P, S // P, SH // P   # 4, 2, 16, 2
TRN2_NC_BASE = (0, 1, 2, 3, 6, 7, 4, 5)
RDESTS = [(0, TRN2_NC_BASE[s]) for s in range(C)]   # relative, self included
RSEM_TARGET = C * (16 // C)                          # 8 senders x 2

# packed-constants layout (one [P, PACKW] f32 DMA): see _pack() below
COL_BQ1, COL_BQ2 = 0, NK
COL_ONESCOL = 2 * NK
COL_NEGHALF = 2 * NK + 1
COL_CID = 2 * NK + 2
COL_IDENT = 2 * NK + 3
COL_BV2 = COL_IDENT + P          # row 0 only
COL_ONESROW = COL_BV2 + D        # row 0 only
PACKW = COL_ONESROW + P

_cache = {}


def _build():
    import concourse.bass as bass
    import concourse.tile as tile
    from concourse import mybir, bacc
    from bass_rust import add_dep_helper, RuntimeValue

    F32 = mybir.dt.float32
    F32R = mybir.dt.float32r
    BF16 = mybir.dt.bfloat16
    I32 = mybir.dt.int32
    AF = mybir.ActivationFunctionType
    ts = bass.ts

    nc = bacc.Bacc()

    ins = {}
    for name, shape, dt in [
        ("x0", [S, D], F32), ("xTq", [D, SH], F32),
        ("Wk1", [D, K], F32), ("Wq1", [D, K], F32), ("Wk2", [D, K], F32),
        ("Wq2", [D, K], F32), ("Wv2", [D, D], F32),
        ("pack", [P, PACKW], F32),
    ]:
        ins[name] = nc.dram_tensor(name, shape, dt, kind="ExternalInput")
    out_ext = nc.dram_tensor("out", [D], F32, kind="ExternalOutput")

    gates = []  # (nop instruction, semaphore, target) -> wait injected post-schedule

    with tile.TileContext(nc) as tc:
        with tc.tile_pool(name="const", bufs=1) as cw, \
             tc.tile_pool(name="big", bufs=1) as big, \
             tc.tile_pool(name="work", bufs=1) as wk, \
             tc.tile_pool(name="send", bufs=1) as snd, \
             tc.tile_pool(name="gath", bufs=1) as gth, \
             tc.tile_pool(name="pp", bufs=2) as pp, \
             tc.tile_pool(name="small", bufs=2) as sm, \
             tc.tile_pool(name="ps", bufs=1, space="PSUM") as ps:

            rsem_k1 = nc.alloc_semaphore("rsem_k1")
            rsem_k2 = nc.alloc_semaphore("rsem_k2")
            rsem_v2 = nc.alloc_semaphore("rsem_v2")
            rsem_hl = nc.alloc_semaphore("rsem_hl")
            rsem_ol = nc.alloc_semaphore("rsem_ol")
            lsem = nc.alloc_semaphore("lsem")

            # ---- input loads ----
            # f32r tiles are loaded with a bitcast (same bits); SP carries
            # xTq + x0, Act carries the weights + the packed constants.
            # Pool stays free for remote-DMA desc generation.
            xTq_r = cw.tile([P, ND, SH], F32R)
            nc.sync.dma_start(xTq_r[:],
                              ins["xTq"][:].bitcast(F32R).rearrange("(k p) j -> p k j", p=P))
            x0_r = big.tile([P, NS, D], F32R, tag="XV")
            for cb in range(4):
                nc.sync.dma_start(
                    x0_r[:, 4 * cb:4 * cb + 4, :],
                    ins["x0"][:].bitcast(F32R).rearrange("(n p) d -> p n d", p=P)[:, 4 * cb:4 * cb + 4, :])
            W_r = {}
            for w, ncol in [("Wk1", K), ("Wq1", K), ("Wk2", K), ("Wq2", K),
                            ("Wv2", D)]:
                W_r[w] = cw.tile([P, ND, ncol], F32R, name=f"W_{w}", tag=f"W_{w}")
                nc.scalar.dma_start(
                    W_r[w][:], ins[w][:].bitcast(F32R).rearrange("(k p) n -> p k n", p=P))
            pk = cw.tile([P, PACKW], F32)
            nc.scalar.dma_start(pk[:], ins["pack"][:])
            bq1_sb = pk[:, COL_BQ1:COL_BQ1 + NK]
            bq2_sb = pk[:, COL_BQ2:COL_BQ2 + NK]
            onescol_f = pk[:, COL_ONESCOL:COL_ONESCOL + 1]
            neghalf = pk[:, COL_NEGHALF:COL_NEGHALF + 1]
            ident_r = pk[:, COL_IDENT:COL_IDENT + P].bitcast(F32R)
            bv2_r = pk[0:1, COL_BV2:COL_BV2 + D].bitcast(F32R)
            ones_r = pk[0:1, COL_ONESROW:COL_ONESROW + P].bitcast(F32R)

            cid_reg = nc.gpsimd.alloc_register("cid")
            nc.gpsimd.reg_load(cid_reg, pk[0:1, COL_CID:COL_CID + 1].bitcast(I32))
            cid_val = RuntimeValue(cid_reg, min_val=0, max_val=C - 1)

            triggers = []

            def bcast_send(full_tile, src_tile, rsem, name):
                """Switch on core id; core j broadcasts src into slot j of
                full_tile on all 8 cores (self included)."""
                for j in tc.Switch(cid_val, C, hint=f"ag_{name}"):
                    nc.gpsimd.remote_dma_broadcast(
                        full_tile[:, j], src_tile[:],
                        remote_sem=rsem, local_sem=lsem, rdests=RDESTS)
                    triggers.append(nc.gpsimd.trigger_dma(count=None))

            def make_gate(rsem, name):
                """Pool nop that (post-scheduling) waits for all 8 broadcasts.
                Ordered after every trigger emitted so far, so a blocked gate
                never delays a send."""
                gate = nc.gpsimd.nop(nofuse=True, hint=f"gate_{name}")
                for t in triggers:
                    add_dep_helper(gate.ins, t.ins, sync=False,
                                   reason="sends before gate")
                gates.append((gate, rsem, RSEM_TARGET))
                return gate

            # ---- block 1 projections (sharded k1) ----
            k1s = snd.tile([P, NK, SH], F32R, tag="snd_k1")
            for m in range(NK):
                pm = ps.tile([P, SH], F32, tag="mm")
                for k in range(ND):
                    nc.tensor.matmul(pm[:], W_r["Wk1"][:, k, ts(m, P)], xTq_r[:, k, :],
                                     start=(k == 0), stop=(k == ND - 1))
                nc.vector.tensor_copy(k1s[:, m, :], pm[:])
            k1_full = gth.tile([P, C, NK, SH], F32R, tag="g_k1", name="dbg_k1full")
            bcast_send(k1_full, k1s, rsem_k1, "k1")
            gate_k1 = make_gate(rsem_k1, "k1")

            q1T = wk.tile([P, NK, SH], F32R, tag="qT1", name="dbg_q1T")
            for m in range(NK):
                pm = ps.tile([P, SH], F32, tag="mm")
                for k in range(ND):
                    nc.tensor.matmul(pm[:], W_r["Wq1"][:, k, ts(m, P)], xTq_r[:, k, :],
                                     start=(k == 0), stop=(k == ND - 1))
                nc.vector.tensor_scalar_add(q1T[:, m, :], pm[:], bq1_sb[:, m:m + 1])

            def attention(qT, kfull, v_blocks, out_dst, pt_dtype,
                          score_gate=None, av_gate=None):
                """out_dst[:, qm, :] = softmax(q.k^T) @ V for this core's
                queries. Keys are in slot-major (= linear) order."""
                for qm in range(NSH):
                    sc = ps.tile([P, S], F32, tag="sc")
                    mx4 = sm.tile([P, 4], F32, tag="mx4")
                    for ks in range(4):
                        for jj in range(2):
                            j = 2 * ks + jj
                            for dm in range(NK):
                                mm = nc.tensor.matmul(
                                    sc[:, j * SH:(j + 1) * SH],
                                    qT[:, dm, ts(qm, P)], kfull[:, j, dm, :],
                                    start=(dm == 0), stop=(dm == NK - 1))
                                if score_gate is not None:
                                    add_dep_helper(mm.ins, score_gate.ins,
                                                   sync=True, reason="gathered keys")
                        nc.vector.reduce_max(mx4[:, ks:ks + 1], sc[:, ts(ks, 512)],
                                             axis=mybir.AxisListType.X)
                    mx = sm.tile([P, 1], F32, tag="mx")
                    nc.vector.reduce_max(mx[:], mx4[:], axis=mybir.AxisListType.X)
                    nm = sm.tile([P, 1], F32, tag="nm")
                    nc.vector.tensor_scalar_mul(nm[:], mx[:], -1.0)
                    Pt = pp.tile([P, S], F32R, tag="P")
                    lsum = sm.tile([P, 4], F32, tag="lsum")
                    for ks in range(4):
                        nc.scalar.activation(Pt[:, ts(ks, 512)], sc[:, ts(ks, 512)],
                                             AF.Exp, bias=nm[:],
                                             accum_out=lsum[:, ks:ks + 1])
                    l = sm.tile([P, 1], F32, tag="l")
                    nc.vector.reduce_sum(l[:], lsum[:], axis=mybir.AxisListType.X)
                    rl = sm.tile([P, 1], F32, tag="rl")
                    nc.vector.reciprocal(rl[:], l[:])
                    PT = pp.tile([P, NS, P], pt_dtype, tag="PT")
                    for g in range(4):
                        tp = ps.tile([P, 4, P], F32R, tag="tp")
                        for u in range(4):
                            nc.tensor.transpose(tp[:, u, :], Pt[:, ts(4 * g + u, P)],
                                                ident_r)
                        nc.vector.tensor_copy(
                            PT[:, 4 * g:4 * g + 4, :].rearrange("p a b -> p (a b)"),
                            tp[:].rearrange("p a b -> p (a b)"))
                    av = ps.tile([P, D], F32, tag="mm")
                    for n in range(NS):
                        mm = nc.tensor.matmul(av[:], PT[:, n, :], v_blocks[n],
                                              start=(n == 0), stop=(n == NS - 1))
                        if av_gate is not None:
                            add_dep_helper(mm.ins, av_gate.ins, sync=True,
                                           reason="gathered values")
                    nc.scalar.activation(out_dst[:, qm, :], av[:], AF.Copy, scale=rl[:])

            out1 = wk.tile([P, NSH, D], F32R, tag="H", name="dbg_out1")
            attention(q1T, k1_full, [x0_r[:, n, :] for n in range(NS)], out1, F32R,
                      score_gate=gate_k1)

            def transpose_rows(src, hl_out=None):
                """src [P, NSH, D] -> dst [P, ND, SH]; optionally extract the
                last row (query SH-1) into hl_out [P, ND]."""
                dst = wk.tile([P, ND, SH], F32R, tag="HT")
                for qm in reversed(range(NSH)):
                    tp = ps.tile([P, 4, P], F32R, tag="tp")
                    for dm in range(ND):
                        nc.tensor.transpose(tp[:, dm, :], src[:, qm, ts(dm, P)],
                                            ident_r)
                    for dm in range(ND):
                        nc.vector.tensor_copy(dst[:, dm, ts(qm, P)], tp[:, dm, :])
                        if hl_out is not None and qm == NSH - 1:
                            nc.vector.tensor_copy(hl_out[:, dm:dm + 1],
                                                  tp[:, dm, P - 1:P])
                return dst

            out1T = transpose_rows(out1)

            # ---- block 2 shard projections + gathers ----
            k2T = snd.tile([P, NK, SH], BF16, tag="snd_k2")
            for m in range(NK):
                pm = ps.tile([P, SH], F32, tag="mm")
                for k in range(ND):
                    nc.tensor.matmul(pm[:], W_r["Wk2"][:, k, ts(m, P)], out1T[:, k, :],
                                     start=(k == 0), stop=(k == ND - 1))
                nc.vector.tensor_copy(k2T[:, m, :], pm[:])
            k2_full = gth.tile([P, C, NK, SH], BF16, tag="g_k2", name="dbg_k2full")
            bcast_send(k2_full, k2T, rsem_k2, "k2")

            q2T = wk.tile([P, NK, SH], BF16, tag="qT2")
            for m in range(NK):
                pm = ps.tile([P, SH], F32, tag="mm")
                for k in range(ND):
                    nc.tensor.matmul(pm[:], W_r["Wq2"][:, k, ts(m, P)], out1T[:, k, :],
                                     start=(k == 0), stop=(k == ND - 1))
                nc.vector.tensor_scalar_add(q2T[:, m, :], pm[:], bq2_sb[:, m:m + 1])

            def rsqrt_act(dstap, srcap):
                """1/sqrt(s) via exp(-0.5*ln(s)) — stays in one act table."""
                t = sm.tile([P, 1], F32, tag="lnt")
                nc.scalar.activation(t[:], srcap, AF.Ln)
                nc.scalar.activation(dstap, t[:], AF.Exp, scale=neghalf)

            def vproj(hT, out_dtype, pool, tag, normalize):
                """v = h @ Wv2 + bv2 for this core's 256 rows; if normalize,
                rows are L2-normalized, else the raw rows and the 1/|row|
                factors (rn [P, NSH]) are returned separately."""
                v_sb = pool.tile([P, NSH, D], out_dtype, tag=tag)
                rn_t = None if normalize else sm.tile([P, NSH], F32, tag="rn3")
                for r in range(NSH):
                    pm = ps.tile([P, D], F32, tag="mm")
                    for k in range(ND):
                        nc.tensor.matmul(pm[:], hT[:, k, ts(r, P)], W_r["Wv2"][:, k, :],
                                         start=(k == 0), stop=False)
                    nc.tensor.matmul(pm[:], ones_r, bv2_r, start=False, stop=True)
                    scr = sm.tile([P, D], F32, tag="scr")
                    ssum = sm.tile([P, 1], F32, tag="ssum")
                    nc.scalar.activation(scr[:], pm[:], AF.Square, accum_out=ssum[:])
                    if normalize:
                        rn = sm.tile([P, 1], F32, tag="rn")
                        rsqrt_act(rn[:], ssum[:])
                        nc.scalar.activation(v_sb[:, r, :], pm[:], AF.Copy, scale=rn[:])
                    else:
                        rsqrt_act(rn_t[:, r:r + 1], ssum[:])
                        nc.vector.tensor_copy(v_sb[:, r, :], pm[:])
                return v_sb, rn_t

            v2, _ = vproj(out1T, BF16, snd, "snd_v2", normalize=True)
            v2_full = gth.tile([P, C, NSH, D], BF16, tag="g_v2", name="dbg_v2full")
            bcast_send(v2_full, v2, rsem_v2, "v2")
            gate_k2 = make_gate(rsem_k2, "k2")
            gate_v2 = make_gate(rsem_v2, "v2")

            # ---- block 2 attention ----
            v2_blocks = [v2_full[:, n // NSH, n % NSH, :] for n in range(NS)]
            hidden = wk.tile([P, NSH, D], F32R, tag="H", name="dbg_hidden")
            attention(q2T, k2_full, v2_blocks, hidden, BF16,
                      score_gate=gate_k2, av_gate=gate_v2)

            hl_c = snd.tile([P, ND], F32R, tag="snd_hl")
            hT = transpose_rows(hidden, hl_out=hl_c)

            hlg = gth.tile([P, C, ND], F32R, tag="g_hl", name="dbg_hlg")
            bcast_send(hlg, hl_c, rsem_hl, "hl")
            gate_hl = make_gate(rsem_hl, "hl")

            # ---- block 3 (flash-style partials over this core's 256 keys).
            # k3/v3/rn3 only need local data and overlap the hl exchange; the
            # 1/|v| factors are folded into p3 so nothing heavy sits on the
            # post-hl critical path.
            k3T = wk.tile([P, NK, SH], F32R, tag="k3")
            for m in range(NK):
                pm = ps.tile([P, SH], F32, tag="mm")
                for k in range(ND):
                    nc.tensor.matmul(pm[:], W_r["Wk2"][:, k, ts(m, P)], hT[:, k, :],
                                     start=(k == 0), stop=(k == ND - 1))
                nc.vector.tensor_copy(k3T[:, m, :], pm[:])
            v3, rn3 = vproj(hT, F32R, wk, "v3", normalize=False)

            # q3 = Wq2^T @ hidden[-1] + bq2 ; hidden[-1] is core 7's slot
            q3 = sm.tile([P, NK], F32R, tag="q3")
            for fm in range(NK):
                pm = ps.tile([P, 1], F32, tag="mm")
                for dm in range(ND):
                    mm = nc.tensor.matmul(pm[:], W_r["Wq2"][:, dm, ts(fm, P)],
                                          hlg[:, C - 1, dm:dm + 1],
                                          start=(dm == 0), stop=(dm == ND - 1))
                    add_dep_helper(mm.ins, gate_hl.ins, sync=True, reason="hl gather")
                nc.vector.tensor_scalar_add(q3[:, fm:fm + 1], pm[:], bq2_sb[:, fm:fm + 1])

            # s3 (scores for my 256 keys; |s3| small so exp needs no max shift)
            s3p = ps.tile([P, NSH], F32, tag="tp")
            for n in range(NSH):
                for fm in range(NK):
                    nc.tensor.matmul(s3p[:, n:n + 1], k3T[:, fm, ts(n, P)],
                                     q3[:, fm:fm + 1],
                                     start=(fm == 0), stop=(fm == NK - 1))
            p3e = sm.tile([P, NSH], F32, tag="p3e")
            nc.scalar.activation(p3e[:], s3p[:], AF.Exp)
            p3 = sm.tile([P, NSH], F32R, tag="p3")
            nc.vector.tensor_tensor(p3[:], p3e[:], rn3[:], mybir.AluOpType.mult)

            # partial numerator oT [128,4] (d on partitions) + replicated l
            ol_ps = ps.tile([P, ND + 1], F32, tag="mm")
            for dm in range(ND):
                for n in range(NSH):
                    nc.tensor.matmul(ol_ps[:, dm:dm + 1], v3[:, n, ts(dm, P)],
                                     p3[:, n:n + 1],
                                     start=(n == 0), stop=(n == NSH - 1))
            l3p = ps.tile([1, 1], F32, tag="tp")
            for n in range(NSH):
                nc.tensor.matmul(l3p[:], p3e[:, n:n + 1], onescol_f,
                                 start=(n == 0), stop=(n == NSH - 1))
            l3f = sm.tile([1, 1], F32R, tag="l3f")
            nc.vector.tensor_copy(l3f[:], l3p[:])
            nc.tensor.matmul(ol_ps[:, ND:ND + 1], ones_r, l3f[:],
                             start=True, stop=True)
            ol = snd.tile([P, ND + 1], F32, tag="snd_ol")
            nc.vector.tensor_copy(ol[:], ol_ps[:])

            olg = gth.tile([P, C, ND + 1], F32, tag="g_ol", name="dbg_olg")
            bcast_send(olg, ol, rsem_ol, "ol")
            gate_ol = make_gate(rsem_ol, "ol")

            tot = wk.tile([P, ND + 1], F32, tag="tot")
            rs = nc.vector.reduce_sum(tot[:], olg[:].rearrange("p c e -> p e c"),
                                      axis=mybir.AxisListType.X)
            add_dep_helper(rs.ins, gate_ol.ins, sync=True, reason="ol gather")
            rl3 = sm.tile([P, 1], F32, tag="rl3")
            nc.vector.reciprocal(rl3[:], tot[:, ND:ND + 1])
            fin = wk.tile([P, ND], F32, tag="fin")
            nc.vector.tensor_scalar_mul(fin[:], tot[:, 0:ND], rl3[:])
            nc.sync.dma_start(out_ext[:].rearrange("(k p) -> p k", p=P), fin[:])

    for gate, sem, target in gates:
        gate.wait_op(sem, target, "sem-ge")
    nc.finalize()
    return nc


def _pack(c, f):
    pk = np.zeros((P, PACKW), np.float32)
    pk[:, COL_BQ1:COL_BQ1 + NK] = f("bq1").reshape(NK, P).T
    pk[:, COL_BQ2:COL_BQ2 + NK] = f("bq2").reshape(NK, P).T
    pk[:, COL_ONESCOL] = 1.0
    pk[:, COL_NEGHALF] = -0.5
    pk[:, COL_CID] = np.array([c], np.int32).view(np.float32)[0]
    pk[:, COL_IDENT:COL_IDENT + P] = np.eye(P, dtype=np.float32)
    pk[0, COL_BV2:COL_BV2 + D] = f("bv2")
    pk[0, COL_ONESROW:COL_ONESROW + P] = 1.0
    return pk


def kernel(**inputs):
    from concourse.bass_utils import run_bass_kernel_spmd

    f = lambda k: np.ascontiguousarray(np.asarray(inputs[k], dtype=np.float32))
    x0 = f("x")[0]                       # [S, D]; batches 1..7 are dead
    xT = np.ascontiguousarray(x0.T)      # [D, S]
    base = {
        "x0": x0,
        "Wk1": f("Wk1"), "Wq1": f("Wq1"), "Wk2": f("Wk2"), "Wq2": f("Wq2"),
        "Wv2": f("Wv2"),
    }
    in_maps = [
        {**base,
         "xTq": np.ascontiguousarray(xT[:, c * SH:(c + 1) * SH]),
         "pack": _pack(c, f)}
        for c in range(C)
    ]

    if "nc" not in _cache:
        _cache["nc"] = _build()
    res = run_bass_kernel_spmd(_cache["nc"], in_maps, list(range(C)))
    return res.results[0]["out"].astype(np.float32)


if __name__ == "__main__":
    d = np.load("/root/problem/inputs.npz")
    out = kernel(**{k: d[k] for k in d.files})
    ref = np.load("/root/problem/ref_out.npy")
    rel = np.abs(out - ref).max() / np.abs(ref).max()
    print("Relative error:", rel)


# revision 16
# speedup vs baseline: 2.8437x; 1.0867x over previous
"""Trainium2 Bass kernel for nn_ModelAttention2Layers (B=8, S=2048, D=512, K=256).

Only batch 0 matters (the reference returns final[0, -1, :]), so the 2048-query
sequence of batch 0 is sharded across the 8 cores (256 queries each).

All cross-core data movement uses relative-addressed remote_dma_broadcast
(SBUF -> SBUF) instead of collective_compute (15us fixed overhead + 40GB/s
each in the perf model). Each allgather is a tc.Switch on the core id: core j
issues one 8-destination broadcast (self included) whose out slot is j, so
slot j always holds core j's shard (keys stay in linear order):
  - block 1's k1 shards (k1 = Wk1^T x is sharded, not recomputed 8x; this
    also removes the 4MB full-xT load),
  - block 2's k2 and v2 shards,
  - hidden[-1] (core 7's last row; 16B/partition),
  - block 3's flash-style [o|l] partials, summed on every core.
Receive-side ordering: a Pool nop "gate" gets a wait on the remote semaphore
injected AFTER tile scheduling (the scheduling sim cannot satisfy
remotely-incremented semaphores), and every reader of a gathered tile gets an
explicit dependency edge on the gate.

Activation-table discipline: only {Exp, Ln, Square, Copy} are used (one
act-func table -> no 1.3us table reloads); 1/sqrt(s) = exp(-0.5*ln(s)).
Matmuls run in float32r / bf16 (full PE rate); k-projection biases are dropped
(softmax-invariant; they are zero in setup_inputs anyway).
"""
import sys

sys.path.insert(0, "/opt/trn_rl_repo")

import numpy as np

S, D, K, P, C = 2048, 512, 256, 128, 8
SH = S // C          # 256 queries/keys per core
ND, NK, NS, NSH = D // P, K // P, S // P, SH // P   # 4, 2, 16, 2
TRN2_NC_BASE = (0, 1, 2, 3, 6, 7, 4, 5)
RDESTS = [(0, TRN2_NC_BASE[s]) for s in range(C)]   # relative, self included
RSEM_TARGET = C * (16 // C)                          # 8 senders x 2

# packed-constants layout (one [P, PACKW] f32 DMA): see _pack() below
COL_BQ1, COL_BQ2 = 0, NK
COL_ONESCOL = 2 * NK
COL_NEGHALF = 2 * NK + 1
COL_CID = 2 * NK + 2
COL_IDENT = 2 * NK + 3
COL_BV2 = COL_IDENT + P          # row 0 only
COL_ONESROW = COL_BV2 + D        # row 0 only
PACKW = COL_ONESROW + P

_cache = {}


def _steer_act_tables():
    """Make the act-table insertion pass resolve Ln to
    natural_log_exp_and_others (which also holds Exp/Square/Copy) instead of
    first-match natural_log: hide Ln from every other table. Table order and
    indices are unchanged, so the emitted act_func_set_id still addresses the
    real act_info.json entry (which genuinely contains Ln). Returns a restore
    thunk."""
    import concourse.bacc as bacc_mod
    from concourse import mybir

    orig = bacc_mod.get_activation_tables

    def steered(arch):
        tabs = dict(orig(arch))
        ln = mybir.ActivationFunctionType.Ln
        exp = mybir.ActivationFunctionType.Exp
        for name, funcs in tabs.items():
            if ln in funcs and exp not in funcs:
                tabs[name] = funcs - {ln}
        return tabs

    bacc_mod.get_activation_tables = steered
    return lambda: setattr(bacc_mod, "get_activation_tables", orig)


def _build():
    import concourse.bass as bass
    import concourse.tile as tile
    from concourse import mybir, bacc
    from bass_rust import add_dep_helper, RuntimeValue

    F32 = mybir.dt.float32
    F32R = mybir.dt.float32r
    BF16 = mybir.dt.bfloat16
    I32 = mybir.dt.int32
    AF = mybir.ActivationFunctionType
    ts = bass.ts

    nc = bacc.Bacc()

    ins = {}
    for name, shape, dt in [
        ("x0", [S, D], F32), ("xTq", [D, SH], F32),
        ("Wk1", [D, K], F32), ("Wq1", [D, K], F32), ("Wk2", [D, K], F32),
        ("Wq2", [D, K], F32), ("Wv2", [D, D], F32),
        ("pack", [P, PACKW], F32),
    ]:
        ins[name] = nc.dram_tensor(name, shape, dt, kind="ExternalInput")
    out_ext = nc.dram_tensor("out", [D], F32, kind="ExternalOutput")

    gates = []  # (nop instruction, semaphore, target) -> wait injected post-schedule

    with tile.TileContext(nc) as tc:
        with tc.tile_pool(name="const", bufs=1) as cw, \
             tc.tile_pool(name="big", bufs=1) as big, \
             tc.tile_pool(name="work", bufs=1) as wk, \
             tc.tile_pool(name="send", bufs=1) as snd, \
             tc.tile_pool(name="gath", bufs=1) as gth, \
             tc.tile_pool(name="pp", bufs=2) as pp, \
             tc.tile_pool(name="small", bufs=2) as sm, \
             tc.tile_pool(name="ps", bufs=1, space="PSUM") as ps:

            rsem_k1 = nc.alloc_semaphore("rsem_k1")
            rsem_k2 = nc.alloc_semaphore("rsem_k2")
            rsem_v2 = nc.alloc_semaphore("rsem_v2")
            rsem_hl = nc.alloc_semaphore("rsem_hl")
            rsem_ol = nc.alloc_semaphore("rsem_ol")
            lsem = nc.alloc_semaphore("lsem")

            # ---- input loads ----
            # f32r tiles are loaded with a bitcast (same bits); SP carries
            # xTq + x0, Act carries the weights + the packed constants.
            # Pool stays free for remote-DMA desc generation.
            xTq_r = cw.tile([P, ND, SH], F32R)
            nc.sync.dma_start(xTq_r[:],
                              ins["xTq"][:].bitcast(F32R).rearrange("(k p) j -> p k j", p=P))
            x0_r = big.tile([P, NS, D], F32R, tag="XV")
            for cb in range(4):
                nc.sync.dma_start(
                    x0_r[:, 4 * cb:4 * cb + 4, :],
                    ins["x0"][:].bitcast(F32R).rearrange("(n p) d -> p n d", p=P)[:, 4 * cb:4 * cb + 4, :])
            W_r = {}
            for w, ncol in [("Wk1", K), ("Wq1", K), ("Wk2", K), ("Wq2", K),
                            ("Wv2", D)]:
                W_r[w] = cw.tile([P, ND, ncol], F32R, name=f"W_{w}", tag=f"W_{w}")
                nc.scalar.dma_start(
                    W_r[w][:], ins[w][:].bitcast(F32R).rearrange("(k p) n -> p k n", p=P))
            pk = cw.tile([P, PACKW], F32)
            nc.scalar.dma_start(pk[:], ins["pack"][:])
            bq1_sb = pk[:, COL_BQ1:COL_BQ1 + NK]
            bq2_sb = pk[:, COL_BQ2:COL_BQ2 + NK]
            onescol_f = pk[:, COL_ONESCOL:COL_ONESCOL + 1]
            neghalf = pk[:, COL_NEGHALF:COL_NEGHALF + 1]
            ident_r = pk[:, COL_IDENT:COL_IDENT + P].bitcast(F32R)
            bv2_r = pk[0:1, COL_BV2:COL_BV2 + D].bitcast(F32R)
            ones_r = pk[0:1, COL_ONESROW:COL_ONESROW + P].bitcast(F32R)

            cid_reg = nc.gpsimd.alloc_register("cid")
            nc.gpsimd.reg_load(cid_reg, pk[0:1, COL_CID:COL_CID + 1].bitcast(I32))
            cid_val = RuntimeValue(cid_reg, min_val=0, max_val=C - 1)

            triggers = []

            def bcast_send(full_tile, src_tile, rsem, name):
                """Switch on core id; core j broadcasts src into slot j of
                full_tile on all 8 cores (self included)."""
                for j in tc.Switch(cid_val, C, hint=f"ag_{name}"):
                    nc.gpsimd.remote_dma_broadcast(
                        full_tile[:, j], src_tile[:],
                        remote_sem=rsem, local_sem=lsem, rdests=RDESTS)
                    triggers.append(nc.gpsimd.trigger_dma(count=None))

            def make_gate(rsem, name):
                """Pool nop that (post-scheduling) waits for all 8 broadcasts.
                Ordered after every trigger emitted so far, so a blocked gate
                never delays a send."""
                gate = nc.gpsimd.nop(nofuse=True, hint=f"gate_{name}")
                for t in triggers:
                    add_dep_helper(gate.ins, t.ins, sync=False,
                                   reason="sends before gate")
                gates.append((gate, rsem, RSEM_TARGET))
                return gate

            # ---- block 1 projections (sharded k1) ----
            k1s = snd.tile([P, NK, SH], F32R, tag="snd_k1")
            for m in range(NK):
                pm = ps.tile([P, SH], F32, tag="mm")
                for k in range(ND):
                    nc.tensor.matmul(pm[:], W_r["Wk1"][:, k, ts(m, P)], xTq_r[:, k, :],
                                     start=(k == 0), stop=(k == ND - 1))
                nc.vector.tensor_copy(k1s[:, m, :], pm[:])
            k1_full = gth.tile([P, C, NK, SH], F32R, tag="g_k1", name="dbg_k1full")
            bcast_send(k1_full, k1s, rsem_k1, "k1")
            gate_k1 = make_gate(rsem_k1, "k1")

            q1T = wk.tile([P, NK, SH], F32R, tag="qT1", name="dbg_q1T")
            for m in range(NK):
                pm = ps.tile([P, SH], F32, tag="mm")
                for k in range(ND):
                    nc.tensor.matmul(pm[:], W_r["Wq1"][:, k, ts(m, P)], xTq_r[:, k, :],
                                     start=(k == 0), stop=(k == ND - 1))
                nc.vector.tensor_scalar_add(q1T[:, m, :], pm[:], bq1_sb[:, m:m + 1])

            def attention(qT, kfull, v_blocks, out_dst, pt_dtype,
                          score_gate=None, av_gate=None):
                """out_dst[:, qm, :] = softmax(q.k^T) @ V for this core's
                queries. Keys are in slot-major (= linear) order."""
                for qm in range(NSH):
                    sc = ps.tile([P, S], F32, tag="sc")
                    mx4 = sm.tile([P, 4], F32, tag="mx4")
                    for ks in range(4):
                        for jj in range(2):
                            j = 2 * ks + jj
                            for dm in range(NK):
                                mm = nc.tensor.matmul(
                                    sc[:, j * SH:(j + 1) * SH],
                                    qT[:, dm, ts(qm, P)], kfull[:, j, dm, :],
                                    start=(dm == 0), stop=(dm == NK - 1))
                                if score_gate is not None:
                                    add_dep_helper(mm.ins, score_gate.ins,
                                                   sync=True, reason="gathered keys")
                        nc.vector.reduce_max(mx4[:, ks:ks + 1], sc[:, ts(ks, 512)],
                                             axis=mybir.AxisListType.X)
                    mx = sm.tile([P, 1], F32, tag="mx")
                    nc.vector.reduce_max(mx[:], mx4[:], axis=mybir.AxisListType.X)
                    nm = sm.tile([P, 1], F32, tag="nm")
                    nc.vector.tensor_scalar_mul(nm[:], mx[:], -1.0)
                    Pt = pp.tile([P, S], F32R, tag="P")
                    lsum = sm.tile([P, 4], F32, tag="lsum")
                    for ks in range(4):
                        nc.scalar.activation(Pt[:, ts(ks, 512)], sc[:, ts(ks, 512)],
                                             AF.Exp, bias=nm[:],
                                             accum_out=lsum[:, ks:ks + 1])
                    l = sm.tile([P, 1], F32, tag="l")
                    nc.vector.reduce_sum(l[:], lsum[:], axis=mybir.AxisListType.X)
                    rl = sm.tile([P, 1], F32, tag="rl")
                    nc.vector.reciprocal(rl[:], l[:])
                    PT = pp.tile([P, NS, P], pt_dtype, tag="PT")
                    for g in range(4):
                        tp = ps.tile([P, 4, P], F32R, tag="tp")
                        for u in range(4):
                            nc.tensor.transpose(tp[:, u, :], Pt[:, ts(4 * g + u, P)],
                                                ident_r)
                        nc.vector.tensor_copy(
                            PT[:, 4 * g:4 * g + 4, :].rearrange("p a b -> p (a b)"),
                            tp[:].rearrange("p a b -> p (a b)"))
                    av = ps.tile([P, D], F32, tag="mm")
                    for n in range(NS):
                        mm = nc.tensor.matmul(av[:], PT[:, n, :], v_blocks[n],
                                              start=(n == 0), stop=(n == NS - 1))
                        if av_gate is not None:
                            add_dep_helper(mm.ins, av_gate.ins, sync=True,
                                           reason="gathered values")
                    nc.scalar.activation(out_dst[:, qm, :], av[:], AF.Copy, scale=rl[:])

            out1 = wk.tile([P, NSH, D], F32R, tag="H", name="dbg_out1")
            attention(q1T, k1_full, [x0_r[:, n, :] for n in range(NS)], out1, F32R,
                      score_gate=gate_k1)

            def transpose_rows(src, hl_out=None):
                """src [P, NSH, D] -> dst [P, ND, SH]; optionally extract the
                last row (query SH-1) into hl_out [P, ND]."""
                dst = wk.tile([P, ND, SH], F32R, tag="HT")
                for qm in reversed(range(NSH)):
                    tp = ps.tile([P, 4, P], F32R, tag="tp")
                    for dm in range(ND):
                        nc.tensor.transpose(tp[:, dm, :], src[:, qm, ts(dm, P)],
                                            ident_r)
                    for dm in range(ND):
                        nc.vector.tensor_copy(dst[:, dm, ts(qm, P)], tp[:, dm, :])
                        if hl_out is not None and qm == NSH - 1:
                            nc.vector.tensor_copy(hl_out[:, dm:dm + 1],
                                                  tp[:, dm, P - 1:P])
                return dst

            out1T = transpose_rows(out1)

            # ---- block 2 shard projections + gathers ----
            k2T = snd.tile([P, NK, SH], BF16, tag="snd_k2")
            for m in range(NK):
                pm = ps.tile([P, SH], F32, tag="mm")
                for k in range(ND):
                    nc.tensor.matmul(pm[:], W_r["Wk2"][:, k, ts(m, P)], out1T[:, k, :],
                                     start=(k == 0), stop=(k == ND - 1))
                nc.vector.tensor_copy(k2T[:, m, :], pm[:])
            k2_full = gth.tile([P, C, NK, SH], BF16, tag="g_k2", name="dbg_k2full")
            bcast_send(k2_full, k2T, rsem_k2, "k2")

            q2T = wk.tile([P, NK, SH], BF16, tag="qT2")
            for m in range(NK):
                pm = ps.tile([P, SH], F32, tag="mm")
                for k in range(ND):
                    nc.tensor.matmul(pm[:], W_r["Wq2"][:, k, ts(m, P)], out1T[:, k, :],
                                     start=(k == 0), stop=(k == ND - 1))
                nc.vector.tensor_scalar_add(q2T[:, m, :], pm[:], bq2_sb[:, m:m + 1])

            def rsqrt_act(dstap, srcap):
                """1/sqrt(s) via exp(-0.5*ln(s)) — stays in one act table."""
                t = sm.tile([P, 1], F32, tag="lnt")
                nc.scalar.activation(t[:], srcap, AF.Ln)
                nc.scalar.activation(dstap, t[:], AF.Exp, scale=neghalf)

            def vproj(hT, out_dtype, pool, tag, normalize):
                """v = h @ Wv2 + bv2 for this core's 256 rows; if normalize,
                rows are L2-normalized, else the raw rows and the 1/|row|
                factors (rn [P, NSH]) are returned separately."""
                v_sb = pool.tile([P, NSH, D], out_dtype, tag=tag)
                rn_t = None if normalize else sm.tile([P, NSH], F32, tag="rn3")
                for r in range(NSH):
                    pm = ps.tile([P, D], F32, tag="mm")
                    for k in range(ND):
                        nc.tensor.matmul(pm[:], hT[:, k, ts(r, P)], W_r["Wv2"][:, k, :],
                                         start=(k == 0), stop=False)
                    nc.tensor.matmul(pm[:], ones_r, bv2_r, start=False, stop=True)
                    scr = sm.tile([P, D], F32, tag="scr")
                    ssum = sm.tile([P, 1], F32, tag="ssum")
                    nc.scalar.activation(scr[:], pm[:], AF.Square, accum_out=ssum[:])
                    if normalize:
                        rn = sm.tile([P, 1], F32, tag="rn")
                        rsqrt_act(rn[:], ssum[:])
                        nc.scalar.activation(v_sb[:, r, :], pm[:], AF.Copy, scale=rn[:])
                    else:
                        rsqrt_act(rn_t[:, r:r + 1], ssum[:])
                        nc.vector.tensor_copy(v_sb[:, r, :], pm[:])
                return v_sb, rn_t

            v2, _ = vproj(out1T, BF16, snd, "snd_v2", normalize=True)
            v2_full = gth.tile([P, C, NSH, D], BF16, tag="g_v2", name="dbg_v2full")
            bcast_send(v2_full, v2, rsem_v2, "v2")
            gate_k2 = make_gate(rsem_k2, "k2")
            gate_v2 = make_gate(rsem_v2, "v2")

            # ---- block 2 attention ----
            v2_blocks = [v2_full[:, n // NSH, n % NSH, :] for n in range(NS)]
            hidden = wk.tile([P, NSH, D], F32R, tag="H", name="dbg_hidden")
            attention(q2T, k2_full, v2_blocks, hidden, BF16,
                      score_gate=gate_k2, av_gate=gate_v2)

            hl_c = snd.tile([P, ND], F32R, tag="snd_hl")
            hT = transpose_rows(hidden, hl_out=hl_c)

            hlg = gth.tile([P, C, ND], F32R, tag="g_hl", name="dbg_hlg")
            bcast_send(hlg, hl_c, rsem_hl, "hl")
            gate_hl = make_gate(rsem_hl, "hl")

            # ---- block 3 (flash-style partials over this core's 256 keys).
            # k3/v3/rn3 only need local data and overlap the hl exchange; the
            # 1/|v| factors are folded into p3 so nothing heavy sits on the
            # post-hl critical path.
            k3T = wk.tile([P, NK, SH], F32R, tag="k3")
            for m in range(NK):
                pm = ps.tile([P, SH], F32, tag="mm")
                for k in range(ND):
                    nc.tensor.matmul(pm[:], W_r["Wk2"][:, k, ts(m, P)], hT[:, k, :],
                                     start=(k == 0), stop=(k == ND - 1))
                nc.vector.tensor_copy(k3T[:, m, :], pm[:])
            v3, rn3 = vproj(hT, F32R, wk, "v3", normalize=False)

            # q3 = Wq2^T @ hidden[-1] + bq2 ; hidden[-1] is core 7's slot
            q3 = sm.tile([P, NK], F32R, tag="q3")
            for fm in range(NK):
                pm = ps.tile([P, 1], F32, tag="mm")
                for dm in range(ND):
                    mm = nc.tensor.matmul(pm[:], W_r["Wq2"][:, dm, ts(fm, P)],
                                          hlg[:, C - 1, dm:dm + 1],
                                          start=(dm == 0), stop=(dm == ND - 1))
                    add_dep_helper(mm.ins, gate_hl.ins, sync=True, reason="hl gather")
                nc.vector.tensor_scalar_add(q3[:, fm:fm + 1], pm[:], bq2_sb[:, fm:fm + 1])

            # s3 (scores for my 256 keys; |s3| small so exp needs no max shift)
            s3p = ps.tile([P, NSH], F32, tag="tp")
            for n in range(NSH):
                for fm in range(NK):
                    nc.tensor.matmul(s3p[:, n:n + 1], k3T[:, fm, ts(n, P)],
                                     q3[:, fm:fm + 1],
                                     start=(fm == 0), stop=(fm == NK - 1))
            p3e = sm.tile([P, NSH], F32, tag="p3e")
            nc.scalar.activation(p3e[:], s3p[:], AF.Exp)
            p3 = sm.tile([P, NSH], F32R, tag="p3")
            nc.vector.tensor_tensor(p3[:], p3e[:], rn3[:], mybir.AluOpType.mult)

            # partial numerator oT [128,4] (d on partitions) + replicated l
            ol_ps = ps.tile([P, ND + 1], F32, tag="mm")
            for dm in range(ND):
                for n in range(NSH):
                    nc.tensor.matmul(ol_ps[:, dm:dm + 1], v3[:, n, ts(dm, P)],
                                     p3[:, n:n + 1],
                                     start=(n == 0), stop=(n == NSH - 1))
            l3p = ps.tile([1, 1], F32, tag="tp")
            for n in range(NSH):
                nc.tensor.matmul(l3p[:], p3e[:, n:n + 1], onescol_f,
                                 start=(n == 0), stop=(n == NSH - 1))
            l3f = sm.tile([1, 1], F32R, tag="l3f")
            nc.vector.tensor_copy(l3f[:], l3p[:])
            nc.tensor.matmul(ol_ps[:, ND:ND + 1], ones_r, l3f[:],
                             start=True, stop=True)
            ol = snd.tile([P, ND + 1], F32, tag="snd_ol")
            nc.vector.tensor_copy(ol[:], ol_ps[:])

            olg = gth.tile([P, C, ND + 1], F32, tag="g_ol", name="dbg_olg")
            bcast_send(olg, ol, rsem_ol, "ol")
            gate_ol = make_gate(rsem_ol, "ol")

            tot = wk.tile([P, ND + 1], F32, tag="tot")
            rs = nc.vector.reduce_sum(tot[:], olg[:].rearrange("p c e -> p e c"),
                                      axis=mybir.AxisListType.X)
            add_dep_helper(rs.ins, gate_ol.ins, sync=True, reason="ol gather")
            rl3 = sm.tile([P, 1], F32, tag="rl3")
            nc.vector.reciprocal(rl3[:], tot[:, ND:ND + 1])
            fin = wk.tile([P, ND], F32, tag="fin")
            nc.vector.tensor_scalar_mul(fin[:], tot[:, 0:ND], rl3[:])
            nc.sync.dma_start(out_ext[:].rearrange("(k p) -> p k", p=P), fin[:])

    for gate, sem, target in gates:
        gate.wait_op(sem, target, "sem-ge")
    restore = _steer_act_tables()
    try:
        nc.finalize()
    finally:
        restore()
    return nc


def _pack(c, f):
    pk = np.zeros((P, PACKW), np.float32)
    pk[:, COL_BQ1:COL_BQ1 + NK] = f("bq1").reshape(NK, P).T
    pk[:, COL_BQ2:COL_BQ2 + NK] = f("bq2").reshape(NK, P).T
    pk[:, COL_ONESCOL] = 1.0
    pk[:, COL_NEGHALF] = -0.5
    pk[:, COL_CID] = np.array([c], np.int32).view(np.float32)[0]
    pk[:, COL_IDENT:COL_IDENT + P] = np.eye(P, dtype=np.float32)
    pk[0, COL_BV2:COL_BV2 + D] = f("bv2")
    pk[0, COL_ONESROW:COL_ONESROW + P] = 1.0
    return pk


def kernel(**inputs):
    from concourse.bass_utils import run_bass_kernel_spmd

    f = lambda k: np.ascontiguousarray(np.asarray(inputs[k], dtype=np.float32))
    x0 = f("x")[0]                       # [S, D]; batches 1..7 are dead
    xT = np.ascontiguousarray(x0.T)      # [D, S]
    base = {
        "x0": x0,
        "Wk1": f("Wk1"), "Wq1": f("Wq1"), "Wk2": f("Wk2"), "Wq2": f("Wq2"),
        "Wv2": f("Wv2"),
    }
    in_maps = [
        {**base,
         "xTq": np.ascontiguousarray(xT[:, c * SH:(c + 1) * SH]),
         "pack": _pack(c, f)}
        for c in range(C)
    ]

    if "nc" not in _cache:
        _cache["nc"] = _build()
    res = run_bass_kernel_spmd(_cache["nc"], in_maps, list(range(C)))
    return res.results[0]["out"].astype(np.float32)


if __name__ == "__main__":
    d = np.load("/root/problem/inputs.npz")
    out = kernel(**{k: d[k] for k in d.files})
    ref = np.load("/root/problem/ref_out.npy")
    rel = np.abs(out - ref).max() / np.abs(ref).max()
    print("Relative error:", rel)


# revision 17
# speedup vs baseline: 3.5029x; 1.2318x over previous
"""Trainium2 Bass kernel for nn_ModelAttention2Layers (B=8, S=2048, D=512, K=256).

Only batch 0 matters (the reference returns final[0, -1, :]), so the 2048-query
sequence of batch 0 is sharded across the 8 cores (256 queries each).

All cross-core data movement uses relative-addressed remote_dma_broadcast
(SBUF -> SBUF) instead of collective_compute (15us fixed overhead + 40GB/s
each in the perf model). Each allgather is a tc.Switch on the core id: core j
issues one 8-destination broadcast (self included) whose out slot is j, so
slot j always holds core j's shard (keys stay in linear order):
  - block 1's k1 shards (k1 = Wk1^T x is sharded, not recomputed 8x; this
    also removes the 4MB full-xT load),
  - block 2's k2 and v2 shards,
  - hidden[-1] (core 7's last row; 16B/partition),
  - block 3's flash-style [o|l] partials, summed on every core.
Receive-side ordering: a Pool nop "gate" gets a wait on the remote semaphore
injected AFTER tile scheduling (the scheduling sim cannot satisfy
remotely-incremented semaphores), and every reader of a gathered tile gets an
explicit dependency edge on the gate.

Activation-table discipline: only {Exp, Ln, Square, Copy} are used (one
act-func table -> no 1.3us table reloads); 1/sqrt(s) = exp(-0.5*ln(s)).
Matmuls run in float32r / bf16 (full PE rate); k-projection biases are dropped
(softmax-invariant; they are zero in setup_inputs anyway).
"""
import sys

sys.path.insert(0, "/opt/trn_rl_repo")

import numpy as np

S, D, K, P, C = 2048, 512, 256, 128, 8
SH = S // C          # 256 queries/keys per core
ND, NK, NS, NSH = D // P, K // P, S // P, SH // P   # 4, 2, 16, 2
TRN2_NC_BASE = (0, 1, 2, 3, 6, 7, 4, 5)
RDESTS = [(0, TRN2_NC_BASE[s]) for s in range(C)]   # relative, self included
RSEM_TARGET = C * (16 // C)                          # 8 senders x 2

# packed-constants layout (one [P, PACKW] f32 DMA): see _pack() below
COL_BQ1, COL_BQ2 = 0, NK
COL_ONESCOL = 2 * NK
COL_NEGHALF = 2 * NK + 1
COL_CID = 2 * NK + 2
COL_IDENT = 2 * NK + 3
COL_BV2 = COL_IDENT + P          # row 0 only
COL_ONESROW = COL_BV2 + D        # row 0 only
PACKW = COL_ONESROW + P

_cache = {}


def _steer_act_tables():
    """Make the act-table insertion pass resolve Ln to
    natural_log_exp_and_others (which also holds Exp/Square/Copy) instead of
    first-match natural_log: hide Ln from every other table. Table order and
    indices are unchanged, so the emitted act_func_set_id still addresses the
    real act_info.json entry (which genuinely contains Ln). Returns a restore
    thunk."""
    import concourse.bacc as bacc_mod
    from concourse import mybir

    orig = bacc_mod.get_activation_tables

    def steered(arch):
        tabs = dict(orig(arch))
        ln = mybir.ActivationFunctionType.Ln
        exp = mybir.ActivationFunctionType.Exp
        for name, funcs in tabs.items():
            if ln in funcs and exp not in funcs:
                tabs[name] = funcs - {ln}
        return tabs

    bacc_mod.get_activation_tables = steered
    return lambda: setattr(bacc_mod, "get_activation_tables", orig)


def _build():
    import concourse.bass as bass
    import concourse.tile as tile
    from concourse import mybir, bacc
    from bass_rust import add_dep_helper, RuntimeValue

    F32 = mybir.dt.float32
    F32R = mybir.dt.float32r
    BF16 = mybir.dt.bfloat16
    I32 = mybir.dt.int32
    AF = mybir.ActivationFunctionType
    ts = bass.ts

    nc = bacc.Bacc()

    ins = {}
    for name, shape, dt in [
        ("x0", [S, D], F32), ("xTq", [D, SH], F32),
        ("Wk1", [D, K], F32), ("Wq1", [D, K], F32), ("Wk2", [D, K], F32),
        ("Wq2", [D, K], F32), ("Wv2", [D, D], F32),
        ("pack", [P, PACKW], F32),
    ]:
        ins[name] = nc.dram_tensor(name, shape, dt, kind="ExternalInput")
    out_ext = nc.dram_tensor("out", [D], F32, kind="ExternalOutput")

    gates = []  # (nop instruction, semaphore, target) -> wait injected post-schedule

    with tile.TileContext(nc) as tc:
        with tc.tile_pool(name="const", bufs=1) as cw, \
             tc.tile_pool(name="big", bufs=1) as big, \
             tc.tile_pool(name="work", bufs=1) as wk, \
             tc.tile_pool(name="send", bufs=1) as snd, \
             tc.tile_pool(name="gath", bufs=1) as gth, \
             tc.tile_pool(name="pp", bufs=2) as pp, \
             tc.tile_pool(name="small", bufs=2) as sm, \
             tc.tile_pool(name="scp", bufs=2, space="PSUM") as scp, \
             tc.tile_pool(name="mmp", bufs=2, space="PSUM") as mmp, \
             tc.tile_pool(name="tpp", bufs=2, space="PSUM") as tpp:

            rsem_k1 = nc.alloc_semaphore("rsem_k1")
            rsem_k2 = nc.alloc_semaphore("rsem_k2")
            rsem_v2 = nc.alloc_semaphore("rsem_v2")
            rsem_hl = nc.alloc_semaphore("rsem_hl")
            rsem_ol = nc.alloc_semaphore("rsem_ol")
            lsem = nc.alloc_semaphore("lsem")

            # ---- input loads ----
            # f32r tiles are loaded with a bitcast (same bits); SP carries
            # xTq + x0, Act carries the weights + the packed constants.
            # Pool stays free for remote-DMA desc generation.
            xTq_r = cw.tile([P, ND, SH], F32R)
            nc.sync.dma_start(xTq_r[:],
                              ins["xTq"][:].bitcast(F32R).rearrange("(k p) j -> p k j", p=P))
            x0_r = big.tile([P, NS, D], F32R, tag="XV")
            for cb in range(4):
                nc.sync.dma_start(
                    x0_r[:, 4 * cb:4 * cb + 4, :],
                    ins["x0"][:].bitcast(F32R).rearrange("(n p) d -> p n d", p=P)[:, 4 * cb:4 * cb + 4, :])
            pk = cw.tile([P, PACKW], F32)
            nc.scalar.dma_start(pk[:], ins["pack"][:])
            W_r = {}
            for w, ncol in [("Wk1", K), ("Wq1", K), ("Wk2", K), ("Wq2", K),
                            ("Wv2", D)]:
                W_r[w] = cw.tile([P, ND, ncol], F32R, name=f"W_{w}", tag=f"W_{w}")
                nc.scalar.dma_start(
                    W_r[w][:], ins[w][:].bitcast(F32R).rearrange("(k p) n -> p k n", p=P))
            bq1_sb = pk[:, COL_BQ1:COL_BQ1 + NK]
            bq2_sb = pk[:, COL_BQ2:COL_BQ2 + NK]
            onescol_f = pk[:, COL_ONESCOL:COL_ONESCOL + 1]
            neghalf = pk[:, COL_NEGHALF:COL_NEGHALF + 1]
            ident_r = pk[:, COL_IDENT:COL_IDENT + P].bitcast(F32R)
            bv2_r = pk[0:1, COL_BV2:COL_BV2 + D].bitcast(F32R)
            ones_r = pk[0:1, COL_ONESROW:COL_ONESROW + P].bitcast(F32R)

            cid_reg = nc.gpsimd.alloc_register("cid")
            nc.gpsimd.reg_load(cid_reg, pk[0:1, COL_CID:COL_CID + 1].bitcast(I32))
            cid_val = RuntimeValue(cid_reg, min_val=0, max_val=C - 1)

            triggers = []

            def bcast_send(full_tile, src_tile, rsem, name):
                """Switch on core id; core j broadcasts src into slot j of
                full_tile on all 8 cores (self included)."""
                for j in tc.Switch(cid_val, C, hint=f"ag_{name}"):
                    nc.gpsimd.remote_dma_broadcast(
                        full_tile[:, j], src_tile[:],
                        remote_sem=rsem, local_sem=lsem, rdests=RDESTS)
                    triggers.append(nc.gpsimd.trigger_dma(count=None))

            def make_gate(rsem, name):
                """Pool nop that (post-scheduling) waits for all 8 broadcasts.
                Ordered after every trigger emitted so far, so a blocked gate
                never delays a send."""
                gate = nc.gpsimd.nop(nofuse=True, hint=f"gate_{name}")
                for t in triggers:
                    add_dep_helper(gate.ins, t.ins, sync=False,
                                   reason="sends before gate")
                gates.append((gate, rsem, RSEM_TARGET))
                return gate

            # ---- block 1 projections (sharded k1) ----
            k1s = snd.tile([P, NK, SH], F32R, tag="snd_k1")
            for m in range(NK):
                pm = mmp.tile([P, SH], F32, tag="mm")
                for k in range(ND):
                    nc.tensor.matmul(pm[:], W_r["Wk1"][:, k, ts(m, P)], xTq_r[:, k, :],
                                     start=(k == 0), stop=(k == ND - 1))
                nc.vector.tensor_copy(k1s[:, m, :], pm[:])
            k1_full = gth.tile([P, C, NK, SH], F32R, tag="g_k1", name="dbg_k1full")
            bcast_send(k1_full, k1s, rsem_k1, "k1")
            gate_k1 = make_gate(rsem_k1, "k1")

            q1T = wk.tile([P, NK, SH], F32R, tag="qT1", name="dbg_q1T")
            for m in range(NK):
                pm = mmp.tile([P, SH], F32, tag="mm")
                for k in range(ND):
                    nc.tensor.matmul(pm[:], W_r["Wq1"][:, k, ts(m, P)], xTq_r[:, k, :],
                                     start=(k == 0), stop=(k == ND - 1))
                nc.vector.tensor_scalar_add(q1T[:, m, :], pm[:], bq1_sb[:, m:m + 1])

            def attention(qT, kfull, v_blocks, out_dst, pt_dtype,
                          score_gate=None, av_gate=None):
                """out_dst[:, qm, :] = softmax(q.k^T) @ V for this core's
                queries. Keys are in slot-major (= linear) order. Scores live
                in two double-buffered [P, 1024] psum halves so query block
                qm+1 overlaps qm's softmax."""
                for qm in range(NSH):
                    halves = []
                    mx4 = sm.tile([P, 4], F32, tag="mx4")
                    for h in range(2):
                        sch = scp.tile([P, 4, SH], F32, tag="sc")
                        halves.append(sch)
                        for jj in range(4):
                            j = 4 * h + jj
                            for dm in range(NK):
                                mm = nc.tensor.matmul(
                                    sch[:, jj, :],
                                    qT[:, dm, ts(qm, P)], kfull[:, j, dm, :],
                                    start=(dm == 0), stop=(dm == NK - 1))
                                if score_gate is not None:
                                    add_dep_helper(mm.ins, score_gate.ins,
                                                   sync=True, reason="gathered keys")
                        for c2 in range(2):
                            nc.vector.reduce_max(
                                mx4[:, 2 * h + c2:2 * h + c2 + 1],
                                sch[:].rearrange("p a b -> p (a b)")[:, ts(c2, 512)],
                                axis=mybir.AxisListType.X)
                    mx = sm.tile([P, 1], F32, tag="mx")
                    nc.vector.reduce_max(mx[:], mx4[:], axis=mybir.AxisListType.X)
                    nm = sm.tile([P, 1], F32, tag="nm")
                    nc.vector.tensor_scalar_mul(nm[:], mx[:], -1.0)
                    Pt = pp.tile([P, S], F32R, tag="P")
                    lsum = sm.tile([P, 4], F32, tag="lsum")
                    for h in range(2):
                        flat = halves[h][:].rearrange("p a b -> p (a b)")
                        for c2 in range(2):
                            nc.scalar.activation(
                                Pt[:, h * 1024 + c2 * 512:h * 1024 + (c2 + 1) * 512],
                                flat[:, ts(c2, 512)], AF.Exp, bias=nm[:],
                                accum_out=lsum[:, 2 * h + c2:2 * h + c2 + 1])
                    l = sm.tile([P, 1], F32, tag="l")
                    nc.vector.reduce_sum(l[:], lsum[:], axis=mybir.AxisListType.X)
                    rl = sm.tile([P, 1], F32, tag="rl")
                    nc.vector.reciprocal(rl[:], l[:])
                    PT = pp.tile([P, NS, P], pt_dtype, tag="PT")
                    for g in range(4):
                        tp = tpp.tile([P, 4, P], F32R, tag="tp")
                        for u in range(4):
                            nc.tensor.transpose(tp[:, u, :], Pt[:, ts(4 * g + u, P)],
                                                ident_r)
                        nc.vector.tensor_copy(
                            PT[:, 4 * g:4 * g + 4, :].rearrange("p a b -> p (a b)"),
                            tp[:].rearrange("p a b -> p (a b)"))
                    av = mmp.tile([P, D], F32, tag="mm")
                    for n in range(NS):
                        mm = nc.tensor.matmul(av[:], PT[:, n, :], v_blocks[n],
                                              start=(n == 0), stop=(n == NS - 1))
                        if av_gate is not None:
                            add_dep_helper(mm.ins, av_gate.ins, sync=True,
                                           reason="gathered values")
                    nc.scalar.activation(out_dst[:, qm, :], av[:], AF.Copy, scale=rl[:])

            out1 = wk.tile([P, NSH, D], F32R, tag="H", name="dbg_out1")
            attention(q1T, k1_full, [x0_r[:, n, :] for n in range(NS)], out1, F32R,
                      score_gate=gate_k1)

            def transpose_rows(src, hl_out=None):
                """src [P, NSH, D] -> dst [P, ND, SH]; optionally extract the
                last row (query SH-1) into hl_out [P, ND]."""
                dst = wk.tile([P, ND, SH], F32R, tag="HT")
                for qm in reversed(range(NSH)):
                    tp = tpp.tile([P, 4, P], F32R, tag="tp")
                    for dm in range(ND):
                        nc.tensor.transpose(tp[:, dm, :], src[:, qm, ts(dm, P)],
                                            ident_r)
                    for dm in range(ND):
                        nc.vector.tensor_copy(dst[:, dm, ts(qm, P)], tp[:, dm, :])
                        if hl_out is not None and qm == NSH - 1:
                            nc.vector.tensor_copy(hl_out[:, dm:dm + 1],
                                                  tp[:, dm, P - 1:P])
                return dst

            out1T = transpose_rows(out1)

            # ---- block 2 shard projections + gathers ----
            k2T = snd.tile([P, NK, SH], BF16, tag="snd_k2")
            for m in range(NK):
                pm = mmp.tile([P, SH], F32, tag="mm")
                for k in range(ND):
                    nc.tensor.matmul(pm[:], W_r["Wk2"][:, k, ts(m, P)], out1T[:, k, :],
                                     start=(k == 0), stop=(k == ND - 1))
                nc.vector.tensor_copy(k2T[:, m, :], pm[:])
            k2_full = gth.tile([P, C, NK, SH], BF16, tag="g_k2", name="dbg_k2full")
            bcast_send(k2_full, k2T, rsem_k2, "k2")

            q2T = wk.tile([P, NK, SH], BF16, tag="qT2")
            for m in range(NK):
                pm = mmp.tile([P, SH], F32, tag="mm")
                for k in range(ND):
                    nc.tensor.matmul(pm[:], W_r["Wq2"][:, k, ts(m, P)], out1T[:, k, :],
                                     start=(k == 0), stop=(k == ND - 1))
                nc.vector.tensor_scalar_add(q2T[:, m, :], pm[:], bq2_sb[:, m:m + 1])

            def rsqrt_act(dstap, srcap):
                """1/sqrt(s) via exp(-0.5*ln(s)) — stays in one act table."""
                t = sm.tile([P, 1], F32, tag="lnt")
                nc.scalar.activation(t[:], srcap, AF.Ln)
                nc.scalar.activation(dstap, t[:], AF.Exp, scale=neghalf)

            def vproj(hT, out_dtype, pool, tag, normalize):
                """v = h @ Wv2 + bv2 for this core's 256 rows; if normalize,
                rows are L2-normalized, else the raw rows and the 1/|row|
                factors (rn [P, NSH]) are returned separately."""
                v_sb = pool.tile([P, NSH, D], out_dtype, tag=tag)
                rn_t = None if normalize else sm.tile([P, NSH], F32, tag="rn3")
                for r in range(NSH):
                    pm = mmp.tile([P, D], F32, tag="mm")
                    for k in range(ND):
                        nc.tensor.matmul(pm[:], hT[:, k, ts(r, P)], W_r["Wv2"][:, k, :],
                                         start=(k == 0), stop=False)
                    nc.tensor.matmul(pm[:], ones_r, bv2_r, start=False, stop=True)
                    scr = sm.tile([P, D], F32, tag="scr")
                    ssum = sm.tile([P, 1], F32, tag="ssum")
                    nc.scalar.activation(scr[:], pm[:], AF.Square, accum_out=ssum[:])
                    if normalize:
                        rn = sm.tile([P, 1], F32, tag="rn")
                        rsqrt_act(rn[:], ssum[:])
                        nc.scalar.activation(v_sb[:, r, :], pm[:], AF.Copy, scale=rn[:])
                    else:
                        rsqrt_act(rn_t[:, r:r + 1], ssum[:])
                        nc.vector.tensor_copy(v_sb[:, r, :], pm[:])
                return v_sb, rn_t

            v2, _ = vproj(out1T, BF16, snd, "snd_v2", normalize=True)
            v2_full = gth.tile([P, C, NSH, D], BF16, tag="g_v2", name="dbg_v2full")
            bcast_send(v2_full, v2, rsem_v2, "v2")
            gate_k2 = make_gate(rsem_k2, "k2")
            gate_v2 = make_gate(rsem_v2, "v2")

            # ---- block 2 attention ----
            v2_blocks = [v2_full[:, n // NSH, n % NSH, :] for n in range(NS)]
            hidden = wk.tile([P, NSH, D], F32R, tag="H", name="dbg_hidden")
            attention(q2T, k2_full, v2_blocks, hidden, BF16,
                      score_gate=gate_k2, av_gate=gate_v2)

            hl_c = snd.tile([P, ND], F32R, tag="snd_hl")
            hT = transpose_rows(hidden, hl_out=hl_c)

            hlg = gth.tile([P, C, ND], F32R, tag="g_hl", name="dbg_hlg")
            bcast_send(hlg, hl_c, rsem_hl, "hl")
            gate_hl = make_gate(rsem_hl, "hl")

            # ---- block 3 (flash-style partials over this core's 256 keys).
            # k3/v3/rn3 only need local data and overlap the hl exchange; the
            # 1/|v| factors are folded into p3 so nothing heavy sits on the
            # post-hl critical path.
            k3T = wk.tile([P, NK, SH], F32R, tag="k3")
            for m in range(NK):
                pm = mmp.tile([P, SH], F32, tag="mm")
                for k in range(ND):
                    nc.tensor.matmul(pm[:], W_r["Wk2"][:, k, ts(m, P)], hT[:, k, :],
                                     start=(k == 0), stop=(k == ND - 1))
                nc.vector.tensor_copy(k3T[:, m, :], pm[:])
            v3, rn3 = vproj(hT, F32R, wk, "v3", normalize=False)

            # q3 = Wq2^T @ hidden[-1] + bq2 ; hidden[-1] is core 7's slot
            q3 = sm.tile([P, NK], F32R, tag="q3")
            for fm in range(NK):
                pm = mmp.tile([P, 1], F32, tag="mm")
                for dm in range(ND):
                    mm = nc.tensor.matmul(pm[:], W_r["Wq2"][:, dm, ts(fm, P)],
                                          hlg[:, C - 1, dm:dm + 1],
                                          start=(dm == 0), stop=(dm == ND - 1))
                    add_dep_helper(mm.ins, gate_hl.ins, sync=True, reason="hl gather")
                nc.vector.tensor_scalar_add(q3[:, fm:fm + 1], pm[:], bq2_sb[:, fm:fm + 1])

            # s3 (scores for my 256 keys; |s3| small so exp needs no max shift)
            s3p = tpp.tile([P, NSH], F32, tag="tp")
            for n in range(NSH):
                for fm in range(NK):
                    nc.tensor.matmul(s3p[:, n:n + 1], k3T[:, fm, ts(n, P)],
                                     q3[:, fm:fm + 1],
                                     start=(fm == 0), stop=(fm == NK - 1))
            p3e = sm.tile([P, NSH], F32, tag="p3e")
            nc.scalar.activation(p3e[:], s3p[:], AF.Exp)
            p3 = sm.tile([P, NSH], F32R, tag="p3")
            nc.vector.tensor_tensor(p3[:], p3e[:], rn3[:], mybir.AluOpType.mult)

            # partial numerator oT [128,4] (d on partitions) + replicated l
            ol_ps = mmp.tile([P, ND + 1], F32, tag="mm")
            for dm in range(ND):
                for n in range(NSH):
                    nc.tensor.matmul(ol_ps[:, dm:dm + 1], v3[:, n, ts(dm, P)],
                                     p3[:, n:n + 1],
                                     start=(n == 0), stop=(n == NSH - 1))
            l3p = tpp.tile([1, 1], F32, tag="tp")
            for n in range(NSH):
                nc.tensor.matmul(l3p[:], p3e[:, n:n + 1], onescol_f,
                                 start=(n == 0), stop=(n == NSH - 1))
            l3f = sm.tile([1, 1], F32R, tag="l3f")
            nc.vector.tensor_copy(l3f[:], l3p[:])
            nc.tensor.matmul(ol_ps[:, ND:ND + 1], ones_r, l3f[:],
                             start=True, stop=True)
            ol = snd.tile([P, ND + 1], F32, tag="snd_ol")
            nc.vector.tensor_copy(ol[:], ol_ps[:])

            olg = gth.tile([P, C, ND + 1], F32, tag="g_ol", name="dbg_olg")
            bcast_send(olg, ol, rsem_ol, "ol")
            gate_ol = make_gate(rsem_ol, "ol")

            tot = wk.tile([P, ND + 1], F32, tag="tot")
            rs = nc.vector.reduce_sum(tot[:], olg[:].rearrange("p c e -> p e c"),
                                      axis=mybir.AxisListType.X)
            add_dep_helper(rs.ins, gate_ol.ins, sync=True, reason="ol gather")
            rl3 = sm.tile([P, 1], F32, tag="rl3")
            nc.vector.reciprocal(rl3[:], tot[:, ND:ND + 1])
            fin = wk.tile([P, ND], F32, tag="fin")
            nc.vector.tensor_scalar_mul(fin[:], tot[:, 0:ND], rl3[:])
            nc.sync.dma_start(out_ext[:].rearrange("(k p) -> p k", p=P), fin[:])

    for gate, sem, target in gates:
        gate.wait_op(sem, target, "sem-ge")
    restore = _steer_act_tables()
    try:
        nc.finalize()
    finally:
        restore()
    return nc


def _pack(c, f):
    pk = np.zeros((P, PACKW), np.float32)
    pk[:, COL_BQ1:COL_BQ1 + NK] = f("bq1").reshape(NK, P).T
    pk[:, COL_BQ2:COL_BQ2 + NK] = f("bq2").reshape(NK, P).T
    pk[:, COL_ONESCOL] = 1.0
    pk[:, COL_NEGHALF] = -0.5
    pk[:, COL_CID] = np.array([c], np.int32).view(np.float32)[0]
    pk[:, COL_IDENT:COL_IDENT + P] = np.eye(P, dtype=np.float32)
    pk[0, COL_BV2:COL_BV2 + D] = f("bv2")
    pk[0, COL_ONESROW:COL_ONESROW + P] = 1.0
    return pk


def kernel(**inputs):
    from concourse.bass_utils import run_bass_kernel_spmd

    f = lambda k: np.ascontiguousarray(np.asarray(inputs[k], dtype=np.float32))
    x0 = f("x")[0]                       # [S, D]; batches 1..7 are dead
    xT = np.ascontiguousarray(x0.T)      # [D, S]
    base = {
        "x0": x0,
        "Wk1": f("Wk1"), "Wq1": f("Wq1"), "Wk2": f("Wk2"), "Wq2": f("Wq2"),
        "Wv2": f("Wv2"),
    }
    in_maps = [
        {**base,
         "xTq": np.ascontiguousarray(xT[:, c * SH:(c + 1) * SH]),
         "pack": _pack(c, f)}
        for c in range(C)
    ]

    if "nc" not in _cache:
        _cache["nc"] = _build()
    res = run_bass_kernel_spmd(_cache["nc"], in_maps, list(range(C)))
    return res.results[0]["out"].astype(np.float32)


if __name__ == "__main__":
    d = np.load("/root/problem/inputs.npz")
    out = kernel(**{k: d[k] for k in d.files})
    ref = np.load("/root/problem/ref_out.npy")
    rel = np.abs(out - ref).max() / np.abs(ref).max()
    print("Relative error:", rel)


# revision 19
# speedup vs baseline: 3.9803x; 1.1363x over previous
"""Trainium2 Bass kernel for nn_ModelAttention2Layers (B=8, S=2048, D=512, K=256).

Only batch 0 matters (the reference returns final[0, -1, :]), so the 2048-query
sequence of batch 0 is sharded across the 8 cores (256 queries each).

All cross-core data movement uses relative-addressed remote_dma_broadcast
(SBUF -> SBUF) instead of collective_compute (15us fixed overhead + 40GB/s
each in the perf model). Each allgather is a tc.Switch on the core id: core j
issues one 8-destination broadcast (self included) whose out slot is j, so
slot j always holds core j's shard (keys stay in linear order):
  - block 1's k1 shards (k1 = Wk1^T x is sharded, not recomputed 8x; this
    also removes the 4MB full-xT load),
  - block 2's k2 and v2 shards,
  - hidden[-1] (core 7's last row; 16B/partition),
  - block 3's flash-style [o|l] partials, summed on every core.
Receive-side ordering: a Pool nop "gate" gets a wait on the remote semaphore
injected AFTER tile scheduling (the scheduling sim cannot satisfy
remotely-incremented semaphores), and every reader of a gathered tile gets an
explicit dependency edge on the gate.

Activation-table discipline: only {Exp, Ln, Square, Copy} are used (one
act-func table -> no 1.3us table reloads); 1/sqrt(s) = exp(-0.5*ln(s)).
Matmuls run in float32r / bf16 (full PE rate); k-projection biases are dropped
(softmax-invariant; they are zero in setup_inputs anyway).
"""
import sys

sys.path.insert(0, "/opt/trn_rl_repo")

import numpy as np

S, D, K, P, C = 2048, 512, 256, 128, 8
SH = S // C          # 256 queries/keys per core
ND, NK, NS, NSH = D // P, K // P, S // P, SH // P   # 4, 2, 16, 2
TRN2_NC_BASE = (0, 1, 2, 3, 6, 7, 4, 5)
RDESTS = [(0, TRN2_NC_BASE[s]) for s in range(C)]   # relative, self included
RSEM_TARGET = C * (16 // C)                          # 8 senders x 2

# packed-constants layout (one [P, PACKW] f32 DMA): see _pack() below
COL_BQ1, COL_BQ2 = 0, NK
COL_ONESCOL = 2 * NK
COL_NEGHALF = 2 * NK + 1
COL_CID = 2 * NK + 2
COL_IDENT = 2 * NK + 3
COL_BV2 = COL_IDENT + P          # row 0 only
COL_ONESROW = COL_BV2 + D        # row 0 only
PACKW = COL_ONESROW + P

_cache = {}


def _steer_act_tables():
    """Make the act-table insertion pass resolve Ln to
    natural_log_exp_and_others (which also holds Exp/Square/Copy) instead of
    first-match natural_log: hide Ln from every other table. Table order and
    indices are unchanged, so the emitted act_func_set_id still addresses the
    real act_info.json entry (which genuinely contains Ln). Returns a restore
    thunk."""
    import concourse.bacc as bacc_mod
    from concourse import mybir

    orig = bacc_mod.get_activation_tables

    def steered(arch):
        tabs = dict(orig(arch))
        ln = mybir.ActivationFunctionType.Ln
        exp = mybir.ActivationFunctionType.Exp
        for name, funcs in tabs.items():
            if ln in funcs and exp not in funcs:
                tabs[name] = funcs - {ln}
            elif exp in funcs and ln not in funcs:
                tabs[name] = funcs - {exp}
        return tabs

    bacc_mod.get_activation_tables = steered
    return lambda: setattr(bacc_mod, "get_activation_tables", orig)


def _build():
    import concourse.bass as bass
    import concourse.tile as tile
    from concourse import mybir, bacc
    from bass_rust import add_dep_helper, RuntimeValue

    F32 = mybir.dt.float32
    F32R = mybir.dt.float32r
    BF16 = mybir.dt.bfloat16
    I32 = mybir.dt.int32
    AF = mybir.ActivationFunctionType
    ts = bass.ts

    nc = bacc.Bacc()

    ins = {}
    for name, shape, dt in [
        ("x0", [S, D], F32), ("xTq", [D, SH], F32),
        ("Wk1", [D, K], F32), ("Wq1", [D, K], F32), ("Wk2", [D, K], F32),
        ("Wq2", [D, K], F32), ("Wv2", [D, D], F32),
        ("pack", [P, PACKW], F32),
    ]:
        ins[name] = nc.dram_tensor(name, shape, dt, kind="ExternalInput")
    out_ext = nc.dram_tensor("out", [D], F32, kind="ExternalOutput")

    gates = []  # (nop instruction, semaphore, target) -> wait injected post-schedule

    with tile.TileContext(nc) as tc:
        with tc.tile_pool(name="const", bufs=1) as cw, \
             tc.tile_pool(name="big", bufs=1) as big, \
             tc.tile_pool(name="work", bufs=1) as wk, \
             tc.tile_pool(name="send", bufs=1) as snd, \
             tc.tile_pool(name="gath", bufs=1) as gth, \
             tc.tile_pool(name="pp", bufs=2) as pp, \
             tc.tile_pool(name="small", bufs=2) as sm, \
             tc.tile_pool(name="scp", bufs=2, space="PSUM") as scp, \
             tc.tile_pool(name="mmp", bufs=2, space="PSUM") as mmp, \
             tc.tile_pool(name="tpp", bufs=2, space="PSUM") as tpp:

            rsem_k1 = nc.alloc_semaphore("rsem_k1")
            rsem_k2 = nc.alloc_semaphore("rsem_k2")
            rsem_v2 = nc.alloc_semaphore("rsem_v2")
            rsem_hl = nc.alloc_semaphore("rsem_hl")
            rsem_ol = nc.alloc_semaphore("rsem_ol")
            lsem = nc.alloc_semaphore("lsem")

            # ---- input loads ----
            # f32r tiles are loaded with a bitcast (same bits); SP carries
            # xTq + x0, Act carries the weights + the packed constants.
            # Pool stays free for remote-DMA desc generation.
            xTq_r = cw.tile([P, ND, SH], F32R)
            nc.sync.dma_start(xTq_r[:],
                              ins["xTq"][:].bitcast(F32R).rearrange("(k p) j -> p k j", p=P))
            x0_r = big.tile([P, NS, D], F32R, tag="XV")
            for cb in range(4):
                nc.sync.dma_start(
                    x0_r[:, 4 * cb:4 * cb + 4, :],
                    ins["x0"][:].bitcast(F32R).rearrange("(n p) d -> p n d", p=P)[:, 4 * cb:4 * cb + 4, :])
            pk = cw.tile([P, PACKW], F32)
            nc.scalar.dma_start(pk[:], ins["pack"][:])
            W_r = {}
            for w, ncol in [("Wk1", K), ("Wq1", K), ("Wk2", K), ("Wq2", K),
                            ("Wv2", D)]:
                W_r[w] = cw.tile([P, ND, ncol], F32R, name=f"W_{w}", tag=f"W_{w}")
                nc.scalar.dma_start(
                    W_r[w][:], ins[w][:].bitcast(F32R).rearrange("(k p) n -> p k n", p=P))
            bq1_sb = pk[:, COL_BQ1:COL_BQ1 + NK]
            bq2_sb = pk[:, COL_BQ2:COL_BQ2 + NK]
            onescol_f = pk[:, COL_ONESCOL:COL_ONESCOL + 1]
            neghalf = pk[:, COL_NEGHALF:COL_NEGHALF + 1]
            ident_r = pk[:, COL_IDENT:COL_IDENT + P].bitcast(F32R)
            bv2_r = pk[0:1, COL_BV2:COL_BV2 + D].bitcast(F32R)
            ones_r = pk[0:1, COL_ONESROW:COL_ONESROW + P].bitcast(F32R)

            cid_reg = nc.gpsimd.alloc_register("cid")
            nc.gpsimd.reg_load(cid_reg, pk[0:1, COL_CID:COL_CID + 1].bitcast(I32))
            cid_val = RuntimeValue(cid_reg, min_val=0, max_val=C - 1)

            triggers = []

            def bcast_send(full_tile, src_tile, rsem, name):
                """Switch on core id; core j broadcasts src into slot j of
                full_tile on all 8 cores (self included)."""
                for j in tc.Switch(cid_val, C, hint=f"ag_{name}"):
                    nc.gpsimd.remote_dma_broadcast(
                        full_tile[:, j], src_tile[:],
                        remote_sem=rsem, local_sem=lsem, rdests=RDESTS)
                    triggers.append(nc.gpsimd.trigger_dma(count=None))

            def make_gate(rsem, name):
                """Pool nop that (post-scheduling) waits for all 8 broadcasts.
                Ordered after every trigger emitted so far, so a blocked gate
                never delays a send."""
                gate = nc.gpsimd.nop(nofuse=True, hint=f"gate_{name}")
                for t in triggers:
                    add_dep_helper(gate.ins, t.ins, sync=False,
                                   reason="sends before gate")
                gates.append((gate, rsem, RSEM_TARGET))
                return gate

            # ---- block 1 projections (sharded k1) ----
            k1s = snd.tile([P, NK, SH], F32R, tag="snd_k1")
            for m in range(NK):
                pm = mmp.tile([P, SH], F32, tag="mm")
                for k in range(ND):
                    nc.tensor.matmul(pm[:], W_r["Wk1"][:, k, ts(m, P)], xTq_r[:, k, :],
                                     start=(k == 0), stop=(k == ND - 1))
                nc.vector.tensor_copy(k1s[:, m, :], pm[:])
            k1_full = gth.tile([P, C, NK, SH], F32R, tag="g_k1", name="dbg_k1full")
            bcast_send(k1_full, k1s, rsem_k1, "k1")
            gate_k1 = make_gate(rsem_k1, "k1")

            q1T = wk.tile([P, NK, SH], F32R, tag="qT1", name="dbg_q1T")
            for m in range(NK):
                pm = mmp.tile([P, SH], F32, tag="mm")
                for k in range(ND):
                    nc.tensor.matmul(pm[:], W_r["Wq1"][:, k, ts(m, P)], xTq_r[:, k, :],
                                     start=(k == 0), stop=(k == ND - 1))
                nc.vector.tensor_scalar_add(q1T[:, m, :], pm[:], bq1_sb[:, m:m + 1])

            def attention(qT, kfull, v_blocks, out_dst, pt_dtype,
                          score_gate=None, av_gate=None, pool_reduce=False):
                """out_dst[:, qm, :] = softmax(q.k^T) @ V for this core's
                queries (keys in linear order). The two query blocks are
                software-pipelined through two [P, 1024] psum score slots."""
                def scores(qm, h):
                    sch = scp.tile([P, 4, SH], F32, tag="sc")
                    for jj in range(4):
                        j = 4 * h + jj
                        for dm in range(NK):
                            mm = nc.tensor.matmul(
                                sch[:, jj, :], qT[:, dm, ts(qm, P)],
                                kfull[:, j, dm, :],
                                start=(dm == 0), stop=(dm == NK - 1))
                            if score_gate is not None:
                                add_dep_helper(mm.ins, score_gate.ins,
                                               sync=True, reason="gathered keys")
                    return sch

                def creduce(sch, mx4, h):
                    flat = sch[:].rearrange("p a b -> p (a b)")
                    nc.vector.reduce_max(mx4[:, 2 * h:2 * h + 1], flat[:, 0:512],
                                         axis=mybir.AxisListType.X)
                    nc.vector.reduce_max(mx4[:, 2 * h + 1:2 * h + 2],
                                         flat[:, 512:1024],
                                         axis=mybir.AxisListType.X)

                def neg_max(mx4):
                    mx = sm.tile([P, 1], F32, tag="mx")
                    nc.vector.reduce_max(mx[:], mx4[:], axis=mybir.AxisListType.X)
                    nm = sm.tile([P, 1], F32, tag="nm")
                    nc.vector.tensor_scalar_mul(nm[:], mx[:], -1.0)
                    return nm

                def exp_half(sch, Pt, lsum, nm, h):
                    flat = sch[:].rearrange("p a b -> p (a b)")
                    nc.scalar.activation(Pt[:, h * 1024:(h + 1) * 1024], flat[:],
                                         AF.Exp, bias=nm[:],
                                         accum_out=lsum[:, h:h + 1])

                def transp(Pt, PT, glo, ghi):
                    for g in range(glo, ghi):
                        tp = tpp.tile([P, 4, P], F32R, tag="tp")
                        for u in range(4):
                            nc.tensor.transpose(tp[:, u, :], Pt[:, ts(4 * g + u, P)],
                                                ident_r)
                        nc.vector.tensor_copy(
                            PT[:, 4 * g:4 * g + 4, :].rearrange("p a b -> p (a b)"),
                            tp[:].rearrange("p a b -> p (a b)"))

                def av_out(PT, lsum, qm):
                    l = sm.tile([P, 1], F32, tag="l")
                    nc.vector.reduce_sum(l[:], lsum[:], axis=mybir.AxisListType.X)
                    rl = sm.tile([P, 1], F32, tag="rl")
                    nc.vector.reciprocal(rl[:], l[:])
                    av = mmp.tile([P, D], F32, tag="mm")
                    for n in range(NS):
                        mm = nc.tensor.matmul(av[:], PT[:, n, :], v_blocks[n],
                                              start=(n == 0), stop=(n == NS - 1))
                        if av_gate is not None:
                            add_dep_helper(mm.ins, av_gate.ins, sync=True,
                                           reason="gathered values")
                    nc.scalar.activation(out_dst[:, qm, :], av[:], AF.Copy,
                                         scale=rl[:])

                mx4_0 = sm.tile([P, 4], F32, tag="mx4")
                mx4_1 = sm.tile([P, 4], F32, tag="mx4")
                s0a = scores(0, 0); creduce(s0a, mx4_0, 0)
                s0b = scores(0, 1); creduce(s0b, mx4_0, 1)
                nm0 = neg_max(mx4_0)
                Pt0 = pp.tile([P, S], F32R, tag="P")
                lsum0 = sm.tile([P, 2], F32, tag="lsum")
                exp_half(s0a, Pt0, lsum0, nm0, 0)
                s1a = scores(1, 0); creduce(s1a, mx4_1, 0)
                exp_half(s0b, Pt0, lsum0, nm0, 1)
                PT0 = pp.tile([P, NS, P], pt_dtype, tag="PT")
                transp(Pt0, PT0, 0, 2)
                s1b = scores(1, 1); creduce(s1b, mx4_1, 1)
                transp(Pt0, PT0, 2, 4)
                av_out(PT0, lsum0, 0)
                nm1 = neg_max(mx4_1)
                Pt1 = pp.tile([P, S], F32R, tag="P")
                lsum1 = sm.tile([P, 2], F32, tag="lsum")
                exp_half(s1a, Pt1, lsum1, nm1, 0)
                exp_half(s1b, Pt1, lsum1, nm1, 1)
                PT1 = pp.tile([P, NS, P], pt_dtype, tag="PT")
                transp(Pt1, PT1, 0, 4)
                av_out(PT1, lsum1, 1)

            out1 = wk.tile([P, NSH, D], F32R, tag="H", name="dbg_out1")
            attention(q1T, k1_full, [x0_r[:, n, :] for n in range(NS)], out1, F32R,
                      score_gate=gate_k1, pool_reduce=True)

            def transpose_rows(src, hl_out=None):
                """src [P, NSH, D] -> dst [P, ND, SH]; optionally extract the
                last row (query SH-1) into hl_out [P, ND]."""
                dst = wk.tile([P, ND, SH], F32R, tag="HT")
                for qm in range(NSH):
                    tp = tpp.tile([P, 4, P], F32R, tag="tp")
                    for dm in range(ND):
                        nc.tensor.transpose(tp[:, dm, :], src[:, qm, ts(dm, P)],
                                            ident_r)
                    nc.vector.tensor_copy(dst[:, :, ts(qm, P)], tp[:])
                    if hl_out is not None and qm == NSH - 1:
                        nc.vector.tensor_copy(
                            hl_out[:],
                            tp[:, :, P - 1:P].rearrange("p a b -> p (a b)"))
                return dst

            out1T = transpose_rows(out1)

            # ---- block 2 shard projections + gathers ----
            k2T = snd.tile([P, NK, SH], BF16, tag="snd_k2")
            for m in range(NK):
                pm = mmp.tile([P, SH], F32, tag="mm")
                for k in range(ND):
                    nc.tensor.matmul(pm[:], W_r["Wk2"][:, k, ts(m, P)], out1T[:, k, :],
                                     start=(k == 0), stop=(k == ND - 1))
                nc.vector.tensor_copy(k2T[:, m, :], pm[:])
            k2_full = gth.tile([P, C, NK, SH], BF16, tag="g_k2", name="dbg_k2full")
            bcast_send(k2_full, k2T, rsem_k2, "k2")
            gate_k2 = make_gate(rsem_k2, "k2")

            q2T = wk.tile([P, NK, SH], BF16, tag="qT2")
            for m in range(NK):
                pm = mmp.tile([P, SH], F32, tag="mm")
                for k in range(ND):
                    nc.tensor.matmul(pm[:], W_r["Wq2"][:, k, ts(m, P)], out1T[:, k, :],
                                     start=(k == 0), stop=(k == ND - 1))
                nc.vector.tensor_scalar_add(q2T[:, m, :], pm[:], bq2_sb[:, m:m + 1])

            def rsqrt_act(dstap, srcap):
                """1/sqrt(s) via exp(-0.5*ln(s)) — stays in one act table."""
                t = sm.tile([P, 1], F32, tag="lnt")
                nc.scalar.activation(t[:], srcap, AF.Ln)
                nc.scalar.activation(dstap, t[:], AF.Exp, scale=neghalf)

            def vproj(hT, out_dtype, pool, tag, normalize):
                """v = h @ Wv2 + bv2 for this core's 256 rows; if normalize,
                rows are L2-normalized, else the raw rows and the 1/|row|
                factors (rn [P, NSH]) are returned separately."""
                v_sb = pool.tile([P, NSH, D], out_dtype, tag=tag)
                rn_t = None if normalize else sm.tile([P, NSH], F32, tag="rn3")
                for r in range(NSH):
                    pm = mmp.tile([P, D], F32, tag="mm")
                    for k in range(ND):
                        nc.tensor.matmul(pm[:], hT[:, k, ts(r, P)], W_r["Wv2"][:, k, :],
                                         start=(k == 0), stop=False)
                    nc.tensor.matmul(pm[:], ones_r, bv2_r, start=False, stop=True)
                    scr = sm.tile([P, D], F32, tag="scr")
                    ssum = sm.tile([P, 1], F32, tag="ssum")
                    nc.scalar.activation(scr[:], pm[:], AF.Square, accum_out=ssum[:])
                    if normalize:
                        rn = sm.tile([P, 1], F32, tag="rn")
                        rsqrt_act(rn[:], ssum[:])
                        nc.vector.tensor_scalar_mul(v_sb[:, r, :], pm[:], rn[:])
                    else:
                        rsqrt_act(rn_t[:, r:r + 1], ssum[:])
                        nc.vector.tensor_copy(v_sb[:, r, :], pm[:])
                return v_sb, rn_t

            v2, _ = vproj(out1T, BF16, snd, "snd_v2", normalize=True)
            v2_full = gth.tile([P, C, NSH, D], BF16, tag="g_v2", name="dbg_v2full")
            bcast_send(v2_full, v2, rsem_v2, "v2")
            gate_v2 = make_gate(rsem_v2, "v2")

            # ---- block 2 attention ----
            v2_blocks = [v2_full[:, n // NSH, n % NSH, :] for n in range(NS)]
            hidden = wk.tile([P, NSH, D], F32R, tag="H", name="dbg_hidden")
            attention(q2T, k2_full, v2_blocks, hidden, BF16,
                      score_gate=gate_k2, av_gate=gate_v2)

            hl_c = snd.tile([P, ND], F32R, tag="snd_hl")
            hT = transpose_rows(hidden, hl_out=hl_c)

            hlg = gth.tile([P, C, ND], F32R, tag="g_hl", name="dbg_hlg")
            bcast_send(hlg, hl_c, rsem_hl, "hl")
            gate_hl = make_gate(rsem_hl, "hl")

            # ---- block 3 (flash-style partials over this core's 256 keys).
            # k3/v3/rn3 only need local data and overlap the hl exchange; the
            # 1/|v| factors are folded into p3 so nothing heavy sits on the
            # post-hl critical path.
            k3T = wk.tile([P, NK, SH], F32R, tag="k3")
            for m in range(NK):
                pm = mmp.tile([P, SH], F32, tag="mm")
                for k in range(ND):
                    nc.tensor.matmul(pm[:], W_r["Wk2"][:, k, ts(m, P)], hT[:, k, :],
                                     start=(k == 0), stop=(k == ND - 1))
                nc.vector.tensor_copy(k3T[:, m, :], pm[:])
            v3, rn3 = vproj(hT, F32R, wk, "v3", normalize=False)

            # q3 = Wq2^T @ hidden[-1] + bq2 ; hidden[-1] is core 7's slot
            q3 = sm.tile([P, NK], F32R, tag="q3")
            for fm in range(NK):
                pm = mmp.tile([P, 1], F32, tag="mm")
                for dm in range(ND):
                    mm = nc.tensor.matmul(pm[:], W_r["Wq2"][:, dm, ts(fm, P)],
                                          hlg[:, C - 1, dm:dm + 1],
                                          start=(dm == 0), stop=(dm == ND - 1))
                    add_dep_helper(mm.ins, gate_hl.ins, sync=True, reason="hl gather")
                nc.vector.tensor_scalar_add(q3[:, fm:fm + 1], pm[:], bq2_sb[:, fm:fm + 1])

            # s3 (scores for my 256 keys; |s3| small so exp needs no max shift)
            s3p = tpp.tile([P, NSH], F32, tag="tp")
            for n in range(NSH):
                for fm in range(NK):
                    nc.tensor.matmul(s3p[:, n:n + 1], k3T[:, fm, ts(n, P)],
                                     q3[:, fm:fm + 1],
                                     start=(fm == 0), stop=(fm == NK - 1))
            p3e = sm.tile([P, NSH], F32, tag="p3e")
            nc.scalar.activation(p3e[:], s3p[:], AF.Exp)
            p3 = sm.tile([P, NSH], F32R, tag="p3")
            nc.vector.tensor_tensor(p3[:], p3e[:], rn3[:], mybir.AluOpType.mult)

            # partial numerator oT [128,4] (d on partitions) + replicated l
            ol_ps = mmp.tile([P, ND + 1], F32, tag="mm")
            for dm in range(ND):
                for n in range(NSH):
                    nc.tensor.matmul(ol_ps[:, dm:dm + 1], v3[:, n, ts(dm, P)],
                                     p3[:, n:n + 1],
                                     start=(n == 0), stop=(n == NSH - 1))
            l3p = tpp.tile([1, 1], F32, tag="tp")
            for n in range(NSH):
                nc.tensor.matmul(l3p[:], p3e[:, n:n + 1], onescol_f,
                                 start=(n == 0), stop=(n == NSH - 1))
            l3f = sm.tile([1, 1], F32R, tag="l3f")
            nc.vector.tensor_copy(l3f[:], l3p[:])
            nc.tensor.matmul(ol_ps[:, ND:ND + 1], ones_r, l3f[:],
                             start=True, stop=True)
            ol = snd.tile([P, ND + 1], F32, tag="snd_ol")
            nc.vector.tensor_copy(ol[:], ol_ps[:])

            olg = gth.tile([P, C, ND + 1], F32, tag="g_ol", name="dbg_olg")
            bcast_send(olg, ol, rsem_ol, "ol")
            gate_ol = make_gate(rsem_ol, "ol")

            tot = wk.tile([P, ND + 1], F32, tag="tot")
            rs = nc.vector.reduce_sum(tot[:], olg[:].rearrange("p c e -> p e c"),
                                      axis=mybir.AxisListType.X)
            add_dep_helper(rs.ins, gate_ol.ins, sync=True, reason="ol gather")
            rl3 = sm.tile([P, 1], F32, tag="rl3")
            nc.vector.reciprocal(rl3[:], tot[:, ND:ND + 1])
            fin = wk.tile([P, ND], F32, tag="fin")
            nc.vector.tensor_scalar_mul(fin[:], tot[:, 0:ND], rl3[:])
            nc.sync.dma_start(out_ext[:].rearrange("(k p) -> p k", p=P), fin[:])

    for gate, sem, target in gates:
        gate.wait_op(sem, target, "sem-ge")
    restore = _steer_act_tables()
    try:
        nc.finalize()
    finally:
        restore()
    return nc


def _pack(c, f):
    pk = np.zeros((P, PACKW), np.float32)
    pk[:, COL_BQ1:COL_BQ1 + NK] = f("bq1").reshape(NK, P).T
    pk[:, COL_BQ2:COL_BQ2 + NK] = f("bq2").reshape(NK, P).T
    pk[:, COL_ONESCOL] = 1.0
    pk[:, COL_NEGHALF] = -0.5
    pk[:, COL_CID] = np.array([c], np.int32).view(np.float32)[0]
    pk[:, COL_IDENT:COL_IDENT + P] = np.eye(P, dtype=np.float32)
    pk[0, COL_BV2:COL_BV2 + D] = f("bv2")
    pk[0, COL_ONESROW:COL_ONESROW + P] = 1.0
    return pk


def kernel(**inputs):
    from concourse.bass_utils import run_bass_kernel_spmd

    f = lambda k: np.ascontiguousarray(np.asarray(inputs[k], dtype=np.float32))
    x0 = f("x")[0]                       # [S, D]; batches 1..7 are dead
    xT = np.ascontiguousarray(x0.T)      # [D, S]
    base = {
        "x0": x0,
        "Wk1": f("Wk1"), "Wq1": f("Wq1"), "Wk2": f("Wk2"), "Wq2": f("Wq2"),
        "Wv2": f("Wv2"),
    }
    in_maps = [
        {**base,
         "xTq": np.ascontiguousarray(xT[:, c * SH:(c + 1) * SH]),
         "pack": _pack(c, f)}
        for c in range(C)
    ]

    if "nc" not in _cache:
        _cache["nc"] = _build()
    res = run_bass_kernel_spmd(_cache["nc"], in_maps, list(range(C)))
    return res.results[0]["out"].astype(np.float32)


if __name__ == "__main__":
    d = np.load("/root/problem/inputs.npz")
    out = kernel(**{k: d[k] for k in d.files})
    ref = np.load("/root/problem/ref_out.npy")
    rel = np.abs(out - ref).max() / np.abs(ref).max()
    print("Relative error:", rel)
